# revision 28
# baseline (speedup 1.0000x reference)
"""Trainium2 Bass kernel for quantized-MoE Bottleneck (nn_Bottleneck_37503654429269).

v6 layout:
- Host precomputes quantized activations Xq (bf16 integers) and ships the
  residual x as bf16; no device-side input quantization.
- Offset-128 storage for intermediate quantized activations: the bn affine
  is written by ACT directly as bf16 with +128 folded into the bias, so the
  bf16 output rounding IS the integer rounding; one DVE clamp
  (max 128, min 128+XB) finishes the quantization. Host folds the
  128*rowsum(w) corrections into the next stage's bias (conv2) or the S3
  drain bias (conv3).
- Group-major schedule: g0 conv1->2->3 completes early; its GN apply
  overlaps g1's convs. All conv3 outputs drain to S3 sbuf (bf16) with the
  offset correction applied; bn_stats reads S3 so PSUM recycles fast.
- DMA: priority-ordered on the SP queue (W1/XQ of g0 first); late tensors
  (XR, g1 conv2/conv3 weights) issue from the idle GpSimd queue.
"""

import numpy as np

BITS = (2, 4, 8)
EPS = 1e-5
B, C_IN, H, W = 32, 1024, 14, 14
WIDTH, OUTC = 256, 1024
PIX = H * W  # 196
NCORES = 8
RB = float(2.0 ** 23)

_NC_CACHE = {}


# ----------------------------------------------------------------------------
# Device program
# ----------------------------------------------------------------------------

def _build_nc(group_sizes):
    from contextlib import ExitStack
    import concourse.bacc as bacc
    import concourse.mybir as mybir
    import concourse.tile as tile

    F32 = mybir.dt.float32
    BF16 = mybir.dt.bfloat16
    ALU = mybir.AluOpType
    ACT = mybir.ActivationFunctionType

    NG = len(group_sizes)
    NS = sum(group_sizes)
    assert NS == 4
    slot0 = [sum(group_sizes[:g]) for g in range(NG)]
    groups = [list(range(slot0[g], slot0[g] + group_sizes[g])) for g in range(NG)]
    chunks = {g: [groups[g][i:i + 2] for i in range(0, len(groups[g]), 2)]
              for g in range(NG)}
    # last group should be the smallest (shortest tail)
    NSL = group_sizes[-1]

    nc = bacc.Bacc("TRN2", target_bir_lowering=False, debug=False,
                   num_devices=NCORES)

    # ---- dram tensors
    # xq: quantized activations [128, kt(8), 4*196] bf16 (integers)
    xq_d = nc.dram_tensor("xq", [128, 8, 4 * PIX], BF16, kind="ExternalInput")
    # xr: residual x [128, mo(8), 4*196] bf16
    xr_d = nc.dram_tensor("xr", [128, 8, 4 * PIX], BF16, kind="ExternalInput")
    w1_d = nc.dram_tensor("w1", [NG, 128, 8, 256], BF16, kind="ExternalInput")
    w2_d = nc.dram_tensor("w2", [NG, 128, 9, 2, 256], BF16, kind="ExternalInput")
    w3_d = nc.dram_tensor("w3", [NG, 128, 2, 1024], BF16, kind="ExternalInput")
    # packed per-partition consts:
    # a1[2,NG] b1r[2,NG] a2[2,NG] b2r[2,NG] xb[NG] gnb[8] d3[8,NG]
    NCC = 4 * (2 * NG) + NG + 8 + 8 * NG
    cc_d = nc.dram_tensor("cc", [128, NCC], F32, kind="ExternalInput")
    # row consts: gng[1024] + per-g (c3e[4*ns], c3e2[4*ns]) + gnbx[8*ns per g]
    NGR = 1024 + sum(16 * n for n in group_sizes)
    gr_d = nc.dram_tensor("gr", [1, NGR], F32, kind="ExternalInput")
    out_d = nc.dram_tensor("out", [128, 8, 4 * PIX], BF16, kind="ExternalOutput")

    with tile.TileContext(nc) as tc, ExitStack() as ctx:
        res = ctx.enter_context(tc.tile_pool(name="res", bufs=1))
        rot = ctx.enter_context(tc.tile_pool(name="rot", bufs=6))
        mmp = ctx.enter_context(tc.tile_pool(name="mmp", bufs=6, space="PSUM"))
        smp = ctx.enter_context(tc.tile_pool(name="smp", bufs=1, space="PSUM"))

        # ---- persistent tiles
        CC = res.tile([128, NCC], F32, name="CC", tag="CC")
        o = 0
        A1 = CC[:, o:o + 2 * NG].rearrange("p (m g) -> p m g", m=2); o += 2 * NG
        B1R = CC[:, o:o + 2 * NG].rearrange("p (m g) -> p m g", m=2); o += 2 * NG
        A2 = CC[:, o:o + 2 * NG].rearrange("p (m g) -> p m g", m=2); o += 2 * NG
        B2R = CC[:, o:o + 2 * NG].rearrange("p (m g) -> p m g", m=2); o += 2 * NG
        XB = CC[:, o:o + NG]; o += NG
        GNB = CC[:, o:o + 8]; o += 8
        D3 = CC[:, o:o + 8 * NG].rearrange("p (m g) -> p m g", m=8); o += 8 * NG

        GR = res.tile([1, NGR], F32, name="GR", tag="GR")
        GNG = GR[:, 0:1024]

        # gnbx: per-partition gn_b replicated per sample, [128, 8*ns] per group
        NC2 = sum(8 * n for n in group_sizes)
        cc2_d = nc.dram_tensor("cc2", [128, NC2], F32, kind="ExternalInput")
        CC2 = res.tile([128, NC2], F32, name="CC2", tag="CC2")

        XQ = res.tile([128, 8, 4 * PIX], BF16, name="XQ", tag="XQ")
        XR = res.tile([128, 8, 4 * PIX], BF16, name="XR", tag="XR")
        W1 = [res.tile([128, 8, 256], BF16, name=f"W1_{g}", tag=f"W1_{g}")
              for g in range(NG)]
        W2 = [res.tile([128, 9, 2, 256], BF16, name=f"W2_{g}", tag=f"W2_{g}")
              for g in range(NG)]
        W3 = [res.tile([128, 2, 1024], BF16, name=f"W3_{g}", tag=f"W3_{g}")
              for g in range(NG)]

        # ---- DMA: single SP queue in strict need-order; XQ-g0 split per
        # kt-pair so conv1's psum accumulation can start on the first pair.
        nc.sync.dma_start(out=CC, in_=cc_d.ap())
        nc.sync.dma_start(out=CC2, in_=cc2_d.ap())
        nc.sync.dma_start(out=GR, in_=gr_d.ap())
        nc.sync.dma_start(out=W1[0], in_=w1_d.ap()[0])
        n0 = group_sizes[0] * PIX
        for kp in range(4):
            nc.sync.dma_start(out=XQ[:, 2 * kp:2 * kp + 2, 0:n0],
                              in_=xq_d.ap()[:, 2 * kp:2 * kp + 2, 0:n0])
        nc.sync.dma_start(out=W2[0], in_=w2_d.ap()[0])
        nc.sync.dma_start(out=W3[0], in_=w3_d.ap()[0])
        for g in range(1, NG):
            nc.sync.dma_start(
                out=XQ[:, :, slot0[g] * PIX:(slot0[g] + group_sizes[g]) * PIX],
                in_=xq_d.ap()[:, :, slot0[g] * PIX:(slot0[g] + group_sizes[g]) * PIX])
            nc.sync.dma_start(out=W1[g], in_=w1_d.ap()[g])
        nc.sync.dma_start(out=XR, in_=xr_d.ap())
        for g in range(1, NG):
            nc.sync.dma_start(out=W2[g], in_=w2_d.ap()[g])
            nc.sync.dma_start(out=W3[g], in_=w3_d.ap()[g])

        ONES = res.tile([128, 1], F32, name="ONES", tag="ONES")
        nc.vector.memset(ONES, 1.0)

        # HP padded conv2 inputs, zero ring (Pool memsets, early)
        HP = [[res.tile([128, group_sizes[g], 16, 18], BF16,
                        name=f"HP{kt}_{g}", tag=f"HP{kt}_{g}")
               for g in range(NG)] for kt in range(2)]
        for kt in range(2):
            for g in range(NG):
                nc.gpsimd.memset(HP[kt][g], 128.0)

        Q2 = [[res.tile([128, group_sizes[g] * PIX], BF16,
                        name=f"Q2{kt}_{g}", tag=f"Q2{kt}_{g}")
               for g in range(NG)] for kt in range(2)]
        # S3 sbuf (bf16, offset-corrected conv3 output) for all groups
        S3 = [res.tile([128, 8, group_sizes[g] * PIX], BF16,
                       name=f"S3_{g}", tag=f"S3_{g}")
              for g in range(NG)]

        BST = [res.tile([128, 8 * group_sizes[g] * 8], F32, name=f"BST{g}",
                        tag=f"BST{g}") for g in range(NG)]
        PQ = [None] * NG
        QG = [None] * NG
        OT = [res.tile([128, 8, group_sizes[g] * PIX], BF16,
                       name=f"OT{g}", tag=f"OT{g}") for g in range(NG)]

        def c1_post(g, mo, ch, ps, pool_rr=None):
            nch = len(ch)
            c0 = ch[0] - slot0[g]
            # bf16 store of a*ps + b + 128 rounds to the integer grid in
            # [128, 256); one clamp finishes quantization (offset-128 kept).
            tpr = rot.tile([128, nch * PIX], BF16, name="tpr", tag="tpr")
            nc.scalar.activation(out=tpr, in_=ps, func=ACT.Identity,
                                 bias=B1R[:, mo, g:g + 1],
                                 scale=A1[:, mo, g:g + 1])
            nc.vector.tensor_scalar(
                out=HP[mo][g][:, c0:c0 + nch, 1:15, 2:16],
                in0=tpr.rearrange("p (s y x) -> p s y x", s=nch, y=14),
                scalar1=128.0, scalar2=XB[:, g:g + 1],
                op0=ALU.max, op1=ALU.min)

        def conv1(g, pool_rr=True):
            for mo in range(2):
                for ch in chunks[g]:
                    nch = len(ch)
                    ps = mmp.tile([128, nch * PIX], F32, name="c1ps", tag="mm")
                    for kt in range(8):
                        nc.tensor.matmul(
                            ps,
                            W1[g][:, kt, mo * 128:(mo + 1) * 128],
                            XQ[:, kt, ch[0] * PIX:(ch[0] + nch) * PIX],
                            start=(kt == 0), stop=(kt == 7))
                    c1_post(g, mo, ch, ps, pool_rr)

        def c2_post(g, mo, ch, ps, pool_rr=None):
            nch = len(ch)
            c0 = ch[0] - slot0[g]
            tpr = rot.tile([128, nch * PIX], BF16, name="tpr", tag="tpr")
            nc.scalar.activation(
                out=tpr, in_=ps.rearrange("p s y x -> p (s y x)"),
                func=ACT.Identity,
                bias=B2R[:, mo, g:g + 1], scale=A2[:, mo, g:g + 1])
            nc.vector.tensor_scalar(
                out=Q2[mo][g][:, c0 * PIX:(c0 + nch) * PIX],
                in0=tpr, scalar1=128.0, scalar2=XB[:, g:g + 1],
                op0=ALU.max, op1=ALU.min)

        def conv2(g, pool_rr=True, defer_posts=False, mos=(0, 1)):
            posts = []
            for mo in mos:
                for ch in chunks[g]:
                    nch = len(ch)
                    c0 = ch[0] - slot0[g]
                    ps = mmp.tile([128, nch, 14, 14], F32, name="c2ps",
                                  tag="mm")
                    first = True
                    for ti, (dy, dx) in enumerate(
                            (dy, dx) for dy in range(3) for dx in range(3)):
                        for kt in range(2):
                            nc.tensor.matmul(
                                ps,
                                W2[g][:, ti, kt, mo * 128:(mo + 1) * 128],
                                HP[kt][g][:, c0:c0 + nch,
                                          dy:dy + 14, dx + 1:dx + 15],
                                start=first, stop=(ti == 8 and kt == 1))
                            first = False
                    if defer_posts:
                        posts.append((mo, ch, ps))
                    else:
                        c2_post(g, mo, ch, ps, pool_rr)
            return posts

        def conv3(g):
            """psum -> ACT drain to S3 sbuf bf16 (removing the 128-offset
            contribution via the -D3 bias); bn_stats reads S3."""
            ns = group_sizes[g]
            bstv = BST[g][:, 0:8 * ns * 6].rearrange("p (t c) -> p t c", c=6)
            for mo in range(8):
                for ch in chunks[g]:
                    nch = len(ch)
                    c0 = ch[0] - slot0[g]
                    ps = mmp.tile([128, nch * PIX], F32, name="c3ps", tag="mm")
                    for kt in range(2):
                        nc.tensor.matmul(
                            ps,
                            W3[g][:, kt, mo * 128:(mo + 1) * 128],
                            Q2[kt][g][:, c0 * PIX:(c0 + nch) * PIX],
                            start=(kt == 0), stop=(kt == 1))
                    nc.scalar.activation(
                        out=S3[g][:, mo, c0 * PIX:(c0 + nch) * PIX],
                        in_=ps, func=ACT.Identity,
                        bias=D3[:, mo, g:g + 1], scale=1.0)
                for si in range(ns):
                    nc.vector.bn_stats(
                        out=bstv[:, mo * ns + si:mo * ns + si + 1, :],
                        in_=S3[g][:, mo, si * PIX:(si + 1) * PIX])

        def stats(g):
            ns = group_sizes[g]
            nst = 8 * ns
            # mean^2 columns (cols 1 and 4 of each 6-tuple)
            mvi = BST[g][:, 0:nst * 6].rearrange(
                "p (t h c) -> p t h c", h=2, c=3)[:, :, :, 1]
            msq = BST[g][:, nst * 6:nst * 8].rearrange("p (t h) -> p t h", h=2)
            nc.vector.tensor_tensor(out=msq, in0=mvi, in1=mvi, op=ALU.mult)
            # partition reduce
            red = smp.tile([1, nst * 8], F32, name="red", tag="red")
            nc.tensor.matmul(red, ONES, BST[g], start=True, stop=True)
            Tg = res.tile([1, nst * 8], F32, name=f"T{g}", tag=f"T{g}")
            nc.scalar.activation(out=Tg, in_=red, func=ACT.Copy,
                                 bias=0.0, scale=1.0)
            return Tg

        def chain(g, Tg):
            """mo-parity pair-add + scalar math -> Fv [1, 8*ns]."""
            ns = group_sizes[g]
            nst = 8 * ns
            nsc = 4 * ns
            TB = res.tile([1, 4 * ns * 8], F32, name=f"TB{g}", tag=f"TB{g}")
            tv = Tg[:, 0:nst * 6].rearrange("p (m o s c) -> p m o s c",
                                            m=4, o=2, c=6)
            nc.vector.tensor_tensor(
                out=TB[:, 0:4 * ns * 6].rearrange("p (m s c) -> p m s c",
                                                  m=4, c=6),
                in0=tv[:, :, 0, :, :], in1=tv[:, :, 1, :, :], op=ALU.add)
            mv = Tg[:, nst * 6:nst * 8].rearrange("p (m o s c) -> p m o s c",
                                                  m=4, o=2, c=2)
            nc.vector.tensor_tensor(
                out=TB[:, 4 * ns * 6:4 * ns * 8].rearrange(
                    "p (m s c) -> p m s c", m=4, c=2),
                in0=mv[:, :, 0, :, :], in1=mv[:, :, 1, :, :], op=ALU.add)
            tb6 = TB[:, 0:4 * ns * 6].rearrange("p (t c) -> p t c", c=6)
            tb2 = TB[:, 4 * ns * 6:4 * ns * 8].rearrange("p (t c) -> p t c",
                                                         c=2)
            SC = res.tile([1, nsc * 4], F32, name=f"SC{g}", tag=f"SC{g}")
            scv = SC.rearrange("p (c t) -> p c t", c=4)
            nc.vector.tensor_tensor(out=scv[:, 0, :], in0=tb6[:, :, 1],
                                    in1=tb6[:, :, 4], op=ALU.add)
            nc.vector.tensor_tensor(out=scv[:, 1, :], in0=tb6[:, :, 2],
                                    in1=tb6[:, :, 5], op=ALU.add)
            nc.vector.tensor_tensor(out=scv[:, 2, :], in0=tb2[:, :, 0],
                                    in1=tb2[:, :, 1], op=ALU.add)
            MEAN = rot.tile([1, nsc], F32, name="MEAN", tag=f"MEAN{g}")
            nc.vector.tensor_scalar(out=MEAN, in0=scv[:, 0, :],
                                    scalar1=1.0 / 512, scalar2=None,
                                    op0=ALU.mult)
            E2 = rot.tile([1, nsc], F32, name="E2", tag=f"E2{g}")
            nc.vector.scalar_tensor_tensor(out=E2, in0=scv[:, 2, :],
                                           scalar=98.0, in1=scv[:, 1, :],
                                           op0=ALU.mult, op1=ALU.add)
            nc.vector.tensor_scalar(out=E2, in0=E2,
                                    scalar1=1.0 / (2 * 128 * PIX),
                                    scalar2=None, op0=ALU.mult)
            VAR = rot.tile([1, nsc], F32, name="VAR", tag=f"VAR{g}")
            nc.vector.tensor_tensor(out=VAR, in0=MEAN, in1=MEAN, op=ALU.mult)
            nc.vector.tensor_tensor(out=VAR, in0=E2, in1=VAR, op=ALU.subtract)
            cbase = 1024 + sum(16 * n for n in group_sizes[:g])
            nc.vector.tensor_tensor(out=VAR, in0=VAR,
                                    in1=GR[:, cbase + nsc:cbase + 2 * nsc],
                                    op=ALU.mult)
            nc.vector.tensor_scalar(out=VAR, in0=VAR, scalar1=EPS,
                                    scalar2=None, op0=ALU.add)
            SD = rot.tile([1, nsc], F32, name="SD", tag=f"SD{g}")
            nc.scalar.activation(out=SD, in_=VAR, func=ACT.Sqrt,
                                 bias=0.0, scale=1.0)
            RC = rot.tile([1, nsc], F32, name="RC", tag=f"RC{g}")
            nc.vector.reciprocal(out=RC, in_=SD)
            Fv = res.tile([1, 8 * ns], F32, name=f"F_{g}", tag=f"F_{g}")
            nc.vector.tensor_tensor(out=Fv[:, 0:nsc], in0=RC,
                                    in1=GR[:, cbase:cbase + nsc], op=ALU.mult)
            nc.vector.scalar_tensor_tensor(
                out=Fv[:, nsc:2 * nsc], in0=MEAN, scalar=-1.0,
                in1=Fv[:, 0:nsc], op0=ALU.mult, op1=ALU.mult)
            return Fv

        def pq_outer(g, Fv):
            """P,Q outer products on PE; ACT drain."""
            ns = group_sizes[g]
            pqp = smp.tile([128, 8, 2, ns], F32, name="pqp", tag="pqp")
            fvv = Fv.rearrange("p (k m s) -> p k m s", k=2, m=4)
            for mo in range(8):
                nc.tensor.matmul(
                    pqp[:, mo, :, :],
                    GNG[:, mo * 128:(mo + 1) * 128],
                    fvv[:, :, mo // 2, :],
                    start=(mo == 0), stop=(mo == 7), skip_group_check=True)
            PQ[g] = res.tile([128, 8, 2, ns], F32, name=f"PQ{g}", tag=f"PQ{g}")
            nc.scalar.activation(out=PQ[g], in_=pqp, func=ACT.Copy,
                                 bias=0.0, scale=1.0)

        def qg_make(g):
            ns = group_sizes[g]
            cb = sum(8 * n for n in group_sizes[:g])
            gnbx = CC2[:, cb:cb + 8 * ns].rearrange("p (m s) -> p m s", m=8)
            QG[g] = res.tile([128, 8, ns], F32, name=f"QG{g}", tag=f"QG{g}")
            nc.vector.tensor_tensor(out=QG[g], in0=PQ[g][:, :, 1, :],
                                    in1=gnbx, op=ALU.add)

        VT = [None] * NG

        def apply_affine(g, mos):
            """DVE affine_then_add: V = S3*P + QG + XR per (mo, si)."""
            ns = group_sizes[g]
            if VT[g] is None:
                VT[g] = res.tile([128, 8, ns * PIX], BF16, name=f"VT{g}",
                                 tag=f"VT{g}")
            for mo in mos:
                for si, slot in enumerate(groups[g]):
                    nc.vector.affine_then_add(
                        out=VT[g][:, mo, si * PIX:(si + 1) * PIX],
                        in0=S3[g][:, mo, si * PIX:(si + 1) * PIX],
                        in1=XR[:, mo, slot * PIX:(slot + 1) * PIX],
                        scale=PQ[g][:, mo, 0, si:si + 1],
                        bias=QG[g][:, mo, si:si + 1])

        def apply_relu(g, mos, engine):
            """relu(V) -> OT (one op per mo), DMA out per 4-mo block."""
            ns = group_sizes[g]
            for mo in mos:
                if engine == "dve":
                    nc.vector.tensor_scalar(
                        out=OT[g][:, mo, :], in0=VT[g][:, mo, :],
                        scalar1=0.0, scalar2=None, op0=ALU.max)
                else:
                    nc.scalar.activation(
                        out=OT[g][:, mo, :], in_=VT[g][:, mo, :],
                        func=ACT.Relu, bias=0.0, scale=1.0)
                nc.sync.dma_start(
                    out=out_d.ap()[:, mo,
                                   slot0[g] * PIX:(slot0[g] + ns) * PIX],
                    in_=OT[g][:, mo, :])

        def apply_last(g):
            """Last group (ns==1): DVE affine_then_add from S3 + ACT relu."""
            ns = group_sizes[g]
            slot = groups[g][0]
            for mo in range(8):
                V = rot.tile([128, ns * PIX], BF16, name="V", tag="Vl")
                nc.vector.affine_then_add(
                    out=V,
                    in0=S3[g][:, mo, :],
                    in1=XR[:, mo, slot * PIX:(slot + 1) * PIX],
                    scale=PQ[g][:, mo, 0, 0:1],
                    bias=QG[g][:, mo, 0:1])
                nc.scalar.activation(
                    out=OT[g][:, mo, :], in_=V, func=ACT.Relu,
                    bias=0.0, scale=1.0)
                if mo in (3, 7):
                    nc.sync.dma_start(
                        out=out_d.ap()[:, mo - 3:mo + 1,
                                       slot0[g] * PIX:(slot0[g] + ns) * PIX],
                        in_=OT[g][:, mo - 3:mo + 1, :])

        # ---------------- schedule ----------------
        gl = NG - 1
        if NG == 2:
            conv1(0)
            conv2(0)
            conv3(0)
            Tg0 = stats(0)
            conv1(gl)
            Fv0 = chain(0, Tg0)
            conv2(gl, mos=(0,))
            pq_outer(0, Fv0)
            qg_make(0)
            conv2(gl, mos=(1,))
            apply_affine(0, range(0, 4))
            apply_relu(0, range(0, 4), "act")
            conv3(gl)
            Tg1 = stats(gl)
            Fv1 = chain(gl, Tg1)
            pq_outer(gl, Fv1)
            qg_make(gl)
            apply_last(gl)
            apply_affine(0, range(4, 8))
            apply_relu(0, range(4, 8), "act")
        else:
            # generic fallback (e.g. (2,2) grouping)
            c1_done = set()
            for g in range(NG):
                if g not in c1_done:
                    conv1(g)
                    c1_done.add(g)
                conv2(g)
                conv3(g)
                Tg = stats(g)
                if g == NG - 2:
                    conv1(gl)
                    c1_done.add(gl)
                Fv = chain(g, Tg)
                pq_outer(g, Fv)
                qg_make(g)
                if g == NG - 1 and group_sizes[g] == 1:
                    apply_last(g)
                else:
                    apply_affine(g, range(0, 8))
                    apply_relu(g, range(0, 8), "dve")

    nc.compile()
    return nc


# ----------------------------------------------------------------------------
# Host side
# ----------------------------------------------------------------------------

def _quant_w(w, lv):
    n = max(lv // 2 - 1, 1)
    s = np.float32(np.abs(w).max()) + np.float32(1e-12)
    k = np.round((w.astype(np.float32) / s) * np.float32(n)).astype(np.float32)
    return k, np.float32(s) / np.float32(n)


def _assign_groups(mask):
    mask = np.asarray(mask).astype(np.int64)
    ids = {e: [int(i) for i in np.nonzero(mask == e)[0]] for e in range(3)}
    counts = [len(ids[e]) for e in range(3)]
    if all(c % 2 == 0 for c in counts):
        group_sizes = (2, 2)
        chunks2 = []
        for e in range(3):
            for j in range(0, counts[e], 2):
                chunks2.append((e, ids[e][j:j + 2]))
        assert len(chunks2) == 16
        core_samples = []
        core_experts = []
        for c in range(8):
            (ea, sa), (eb, sb) = chunks2[2 * c], chunks2[2 * c + 1]
            core_samples.append(sa + sb)
            core_experts.append([ea, eb])
        return group_sizes, core_samples, core_experts

    base = [c % 3 for c in counts]
    need = (8 - sum(base)) // 3
    t = [0, 0, 0]
    for e in range(3):
        cap = (counts[e] - base[e]) // 3
        take = min(cap, need)
        t[e] = take
        need -= take
        if need == 0:
            break
    assert need == 0
    b = [base[e] + 3 * t[e] for e in range(3)]
    a = [(counts[e] - b[e]) // 3 for e in range(3)]
    assert sum(a) == 8 and sum(b) == 8
    trip = []
    single = []
    for e in range(3):
        pos = 0
        for _ in range(a[e]):
            trip.append((e, ids[e][pos:pos + 3]))
            pos += 3
        for _ in range(b[e]):
            single.append((e, [ids[e][pos]]))
            pos += 1
        assert pos == counts[e]
    core_samples = []
    core_experts = []
    for c in range(8):
        ea, sa = trip[c]
        eb, sb = single[c]
        core_samples.append(sa + sb)
        core_experts.append([ea, eb])
    return (3, 1), core_samples, core_experts


def kernel(x, mask, w1, w2, w3, bn1_g, bn1_b, bn1_m, bn1_v,
           bn2_g, bn2_b, bn2_m, bn2_v, gn_g, gn_b):
    import ml_dtypes
    from concourse.bass_utils import run_bass_kernel_spmd

    bf16 = ml_dtypes.bfloat16
    f32 = np.float32
    x = np.asarray(x, f32)
    mask = np.asarray(mask)
    w1 = np.asarray(w1, f32)
    w2 = np.asarray(w2, f32)
    w3 = np.asarray(w3, f32)
    bn1 = [np.asarray(v, f32) for v in (bn1_g, bn1_b, bn1_m, bn1_v)]
    bn2 = [np.asarray(v, f32) for v in (bn2_g, bn2_b, bn2_m, bn2_v)]
    gn_g = np.asarray(gn_g, f32)
    gn_b = np.asarray(gn_b, f32)

    group_sizes, core_samples, core_experts = _assign_groups(mask)
    NG = len(group_sizes)

    lv_of = [2 ** b for b in BITS]
    K1, K2, K3 = {}, {}, {}
    CW = {}
    for e in range(3):
        lv = lv_of[e]
        k1, c1 = _quant_w(w1, lv)
        k2, c2 = _quant_w(w2, lv)
        k3, c3 = _quant_w(w3, lv)
        K1[e] = k1.reshape(256, 1024)
        K2[e] = k2.reshape(256, 256, 3, 3)
        K3[e] = k3.reshape(1024, 256)
        CW[e] = (c1, c2, c3)

    inv1 = bn1[0] / np.sqrt(bn1[3] + f32(EPS))
    bb1 = bn1[1] - bn1[2] * inv1
    inv2 = bn2[0] / np.sqrt(bn2[3] + f32(EPS))
    bb2 = bn2[1] - bn2[2] * inv2

    def pack_w(e):
        k1t = K1[e].T.reshape(8, 128, 256).transpose(1, 0, 2)
        k2t = K2[e].transpose(2, 3, 1, 0).reshape(9, 2, 128, 256)
        k2t = k2t.transpose(2, 0, 1, 3)
        k3t = K3[e].T.reshape(2, 128, 1024).transpose(1, 0, 2)
        return (np.ascontiguousarray(k1t).astype(bf16),
                np.ascontiguousarray(k2t).astype(bf16),
                np.ascontiguousarray(k3t).astype(bf16))

    packed = {e: pack_w(e) for e in set(int(v) for v in np.asarray(mask))}

    # host-side input quantization per sample (exact integer grid)
    lv_smp = np.array([lv_of[int(mask[s])] for s in range(B)], f32)
    xq_full = np.clip(np.round(x * (lv_smp - 1)[:, None, None, None]),
                      0.0, (lv_smp - 1)[:, None, None, None]).astype(f32)

    in_maps = []
    for c in range(8):
        sids = core_samples[c]
        experts = core_experts[c]

        # [128, 8, 4*196] channel-tile major
        xqc = xq_full[sids].reshape(4, 8, 128, PIX).transpose(2, 1, 0, 3) \
                           .reshape(128, 8, 4 * PIX)
        xrc = x[sids].reshape(4, 8, 128, PIX).transpose(2, 1, 0, 3) \
                     .reshape(128, 8, 4 * PIX)

        w1c = np.stack([packed[experts[g]][0] for g in range(NG)])
        w2c = np.stack([packed[experts[g]][1] for g in range(NG)])
        w3c = np.stack([packed[experts[g]][2] for g in range(NG)])

        glv = [lv_of[experts[g]] for g in range(NG)]
        NCC = 4 * (2 * NG) + NG + 8 + 8 * NG
        cc = np.zeros((128, NCC), f32)
        a1 = np.zeros((128, 2, NG), f32)
        b1 = np.zeros((128, 2, NG), f32)
        a2 = np.zeros((128, 2, NG), f32)
        b2 = np.zeros((128, 2, NG), f32)
        d3 = np.zeros((128, 8, NG), f32)
        for g in range(NG):
            e = experts[g]
            lv = glv[g]
            c1, c2, c3 = CW[e]
            # offset-128 storage: +128 into the quantizing biases; the
            # 128*rowsum(w) contribution of the offset inputs is removed from
            # the next stage (conv2 bias) or the S3 drain bias (conv3).
            w2sum = K2[e].sum(axis=(1, 2, 3))          # (256,)
            w3sum = K3[e].sum(axis=1)                  # (1024,)
            a1[:, :, g] = (inv1 * c1).reshape(2, 128).T
            b1[:, :, g] = (bb1 * f32(lv - 1)).reshape(2, 128).T + f32(128.0)
            a2[:, :, g] = (inv2 * c2).reshape(2, 128).T
            b2[:, :, g] = (bb2 * f32(lv - 1)
                           - inv2 * c2 * f32(128.0) * w2sum
                           ).reshape(2, 128).T + f32(128.0)
            d3[:, :, g] = (-f32(128.0) * w3sum).reshape(8, 128).T
        o = 0
        cc[:, o:o + 2 * NG] = a1.reshape(128, 2 * NG); o += 2 * NG
        cc[:, o:o + 2 * NG] = b1.reshape(128, 2 * NG); o += 2 * NG
        cc[:, o:o + 2 * NG] = a2.reshape(128, 2 * NG); o += 2 * NG
        cc[:, o:o + 2 * NG] = b2.reshape(128, 2 * NG); o += 2 * NG
        cc[:, o:o + NG] = [128.0 + lv - 1 for lv in glv]; o += NG
        cc[:, o:o + 8] = gn_b.reshape(8, 128).T; o += 8
        cc[:, o:o + 8 * NG] = d3.reshape(128, 8 * NG); o += 8 * NG

        NGR = 1024 + sum(16 * n for n in group_sizes)
        gr = np.zeros((1, NGR), f32)
        gr[0, 0:1024] = gn_g
        off = 1024
        for g in range(NG):
            ns = group_sizes[g]
            e = experts[g]
            lv = glv[g]
            c3e = CW[e][2] / f32(lv - 1)
            gr[0, off:off + 4 * ns] = c3e
            gr[0, off + 4 * ns:off + 8 * ns] = c3e * c3e
            off += 16 * ns

        NC2 = sum(8 * n for n in group_sizes)
        cc2 = np.zeros((128, NC2), f32)
        cb = 0
        gnbp = gn_b.reshape(8, 128).T  # [128, 8]
        for g in range(NG):
            ns = group_sizes[g]
            cc2[:, cb:cb + 8 * ns] = np.repeat(gnbp, ns, axis=1)
            cb += 8 * ns

        in_maps.append({
            "xq": xqc.astype(bf16), "xr": xrc.astype(bf16),
            "w1": w1c, "w2": w2c, "w3": w3c,
            "cc": cc, "gr": gr, "cc2": cc2,
        })

    key = group_sizes
    if key not in _NC_CACHE:
        _NC_CACHE[key] = _build_nc(group_sizes)
    nc = _NC_CACHE[key]

    res = run_bass_kernel_spmd(nc, in_maps, core_ids=list(range(NCORES)))

    out = np.zeros((B, OUTC, H, W), f32)
    for c in range(8):
        oc = np.asarray(res.results[c]["out"], dtype=f32)  # [128, 8, 4*PIX]
        oc = oc.reshape(128, 8, 4, PIX).transpose(2, 1, 0, 3) \
               .reshape(4, OUTC, H, W)
        for t, sid in enumerate(core_samples[c]):
            out[sid] = oc[t]
    return out


# revision 29
# speedup vs baseline: 1.1527x; 1.1527x over previous
"""Trainium2 Bass kernel for quantized-MoE Bottleneck (nn_Bottleneck_37503654429269).

v6 layout:
- Host precomputes quantized activations Xq (bf16 integers) and ships the
  residual x as bf16; no device-side input quantization.
- Offset-128 storage for intermediate quantized activations: the bn affine
  is written by ACT directly as bf16 with +128 folded into the bias, so the
  bf16 output rounding IS the integer rounding; one DVE clamp
  (max 128, min 128+XB) finishes the quantization. Host folds the
  128*rowsum(w) corrections into the next stage's bias (conv2) or the S3
  drain bias (conv3).
- Group-major schedule: g0 conv1->2->3 completes early; its GN apply
  overlaps g1's convs. All conv3 outputs drain to S3 sbuf (bf16) with the
  offset correction applied; bn_stats reads S3 so PSUM recycles fast.
- DMA: priority-ordered on the SP queue (W1/XQ of g0 first); late tensors
  (XR, g1 conv2/conv3 weights) issue from the idle GpSimd queue.
"""

import numpy as np

BITS = (2, 4, 8)
EPS = 1e-5
B, C_IN, H, W = 32, 1024, 14, 14
WIDTH, OUTC = 256, 1024
PIX = H * W  # 196
NCORES = 8
RB = float(2.0 ** 23)

_NC_CACHE = {}


# ----------------------------------------------------------------------------
# Device program
# ----------------------------------------------------------------------------

def _build_nc(group_sizes):
    from contextlib import ExitStack
    import concourse.bacc as bacc
    import concourse.mybir as mybir
    import concourse.tile as tile

    F32 = mybir.dt.float32
    BF16 = mybir.dt.bfloat16
    ALU = mybir.AluOpType
    ACT = mybir.ActivationFunctionType

    NG = len(group_sizes)
    NS = sum(group_sizes)
    assert NS == 4
    slot0 = [sum(group_sizes[:g]) for g in range(NG)]
    groups = [list(range(slot0[g], slot0[g] + group_sizes[g])) for g in range(NG)]
    chunks = {g: [groups[g][i:i + 2] for i in range(0, len(groups[g]), 2)]
              for g in range(NG)}
    # last group should be the smallest (shortest tail)
    NSL = group_sizes[-1]

    nc = bacc.Bacc("TRN2", target_bir_lowering=False, debug=False,
                   num_devices=NCORES)

    # ---- dram tensors
    # xq: quantized activations [128, kt(8), 4*196] bf16 (integers)
    xq_d = nc.dram_tensor("xq", [128, 8, 4 * PIX], BF16, kind="ExternalInput")
    # xr: residual x [128, mo(8), 4*196] bf16
    xr_d = nc.dram_tensor("xr", [128, 8, 4 * PIX], BF16, kind="ExternalInput")
    w1_d = nc.dram_tensor("w1", [NG, 128, 8, 256], BF16, kind="ExternalInput")
    w2_d = nc.dram_tensor("w2", [NG, 128, 9, 2, 256], BF16, kind="ExternalInput")
    w3_d = nc.dram_tensor("w3", [NG, 128, 2, 1024], BF16, kind="ExternalInput")
    # packed per-partition consts:
    # a1[2,NG] b1r[2,NG] a2[2,NG] b2r[2,NG] xb[NG] gnb[8] d3[8,NG]
    NCC = 4 * (2 * NG) + NG + 8 + 8 * NG
    cc_d = nc.dram_tensor("cc", [128, NCC], F32, kind="ExternalInput")
    # row consts: gng[1024] + per-g (c3e[4*ns], c3e2[4*ns]) + gnbx[8*ns per g]
    NGR = 1024 + sum(16 * n for n in group_sizes)
    gr_d = nc.dram_tensor("gr", [1, NGR], F32, kind="ExternalInput")
    out_d = nc.dram_tensor("out", [128, 8, 4 * PIX], BF16, kind="ExternalOutput")

    with tile.TileContext(nc) as tc, ExitStack() as ctx:
        res = ctx.enter_context(tc.tile_pool(name="res", bufs=1))
        rot = ctx.enter_context(tc.tile_pool(name="rot", bufs=4))
        mmp = ctx.enter_context(tc.tile_pool(name="mmp", bufs=5, space="PSUM"))
        smp = ctx.enter_context(tc.tile_pool(name="smp", bufs=1, space="PSUM"))

        # ---- persistent tiles
        CC = res.tile([128, NCC], F32, name="CC", tag="CC")
        o = 0
        A1 = CC[:, o:o + 2 * NG].rearrange("p (m g) -> p m g", m=2); o += 2 * NG
        B1R = CC[:, o:o + 2 * NG].rearrange("p (m g) -> p m g", m=2); o += 2 * NG
        A2 = CC[:, o:o + 2 * NG].rearrange("p (m g) -> p m g", m=2); o += 2 * NG
        B2R = CC[:, o:o + 2 * NG].rearrange("p (m g) -> p m g", m=2); o += 2 * NG
        XB = CC[:, o:o + NG]; o += NG
        GNB = CC[:, o:o + 8]; o += 8
        D3 = CC[:, o:o + 8 * NG].rearrange("p (m g) -> p m g", m=8); o += 8 * NG

        GR = res.tile([1, NGR], F32, name="GR", tag="GR")
        GNG = GR[:, 0:1024]

        # gnbx: per-partition gn_b replicated per sample, [128, 8*ns] per group
        NC2 = sum(8 * n for n in group_sizes)
        cc2_d = nc.dram_tensor("cc2", [128, NC2], F32, kind="ExternalInput")
        CC2 = res.tile([128, NC2], F32, name="CC2", tag="CC2")

        XQ = res.tile([128, 8, 4 * PIX], BF16, name="XQ", tag="XQ")
        XR = res.tile([128, 8, 4 * PIX], BF16, name="XR", tag="XR")
        W1 = [res.tile([128, 8, 256], BF16, name=f"W1_{g}", tag=f"W1_{g}")
              for g in range(NG)]
        W2 = [res.tile([128, 9, 2, 256], BF16, name=f"W2_{g}", tag=f"W2_{g}")
              for g in range(NG)]
        W3 = [res.tile([128, 2, 1024], BF16, name=f"W3_{g}", tag=f"W3_{g}")
              for g in range(NG)]

        # ---- DMA: single SP queue in strict need-order; XQ-g0 split per
        # kt-pair so conv1's psum accumulation can start on the first pair.
        nc.sync.dma_start(out=CC, in_=cc_d.ap())
        nc.sync.dma_start(out=CC2, in_=cc2_d.ap())
        nc.sync.dma_start(out=GR, in_=gr_d.ap())
        nc.sync.dma_start(out=W1[0], in_=w1_d.ap()[0])
        n0 = group_sizes[0] * PIX
        for kp in range(4):
            nc.sync.dma_start(out=XQ[:, 2 * kp:2 * kp + 2, 0:n0],
                              in_=xq_d.ap()[:, 2 * kp:2 * kp + 2, 0:n0])
        nc.sync.dma_start(out=W2[0], in_=w2_d.ap()[0])
        nc.sync.dma_start(out=W3[0], in_=w3_d.ap()[0])
        for g in range(1, NG):
            nc.sync.dma_start(
                out=XQ[:, :, slot0[g] * PIX:(slot0[g] + group_sizes[g]) * PIX],
                in_=xq_d.ap()[:, :, slot0[g] * PIX:(slot0[g] + group_sizes[g]) * PIX])
            nc.sync.dma_start(out=W1[g], in_=w1_d.ap()[g])
        nc.sync.dma_start(out=XR, in_=xr_d.ap())
        for g in range(1, NG):
            nc.sync.dma_start(out=W2[g], in_=w2_d.ap()[g])
            nc.sync.dma_start(out=W3[g], in_=w3_d.ap()[g])

        ONES = res.tile([128, 1], F32, name="ONES", tag="ONES")
        nc.vector.memset(ONES, 1.0)

        # HP padded conv2 inputs, zero ring (Pool memsets, early)
        HP = [[res.tile([128, group_sizes[g], 16, 18], BF16,
                        name=f"HP{kt}_{g}", tag=f"HP{kt}_{g}")
               for g in range(NG)] for kt in range(2)]
        for kt in range(2):
            for g in range(NG):
                nc.gpsimd.memset(HP[kt][g], 128.0)

        Q2 = [[res.tile([128, group_sizes[g] * PIX], BF16,
                        name=f"Q2{kt}_{g}", tag=f"Q2{kt}_{g}")
               for g in range(NG)] for kt in range(2)]
        # S3 sbuf (bf16, offset-corrected conv3 output) for all groups
        S3 = [res.tile([128, 8, group_sizes[g] * PIX], BF16,
                       name=f"S3_{g}", tag=f"S3_{g}")
              for g in range(NG)]

        BST = [res.tile([128, 8 * group_sizes[g] * 8], F32, name=f"BST{g}",
                        tag=f"BST{g}") for g in range(NG)]
        PQ = [None] * NG
        QG = [None] * NG
        OT = [res.tile([128, 8, group_sizes[g] * PIX], BF16,
                       name=f"OT{g}", tag=f"OT{g}") for g in range(NG)]

        def c1_post(g, mo, ch, ps, pool_rr=None):
            nch = len(ch)
            c0 = ch[0] - slot0[g]
            # bf16 store of a*ps + b + 128 rounds to the integer grid in
            # [128, 256); one clamp finishes quantization (offset-128 kept).
            tpr = rot.tile([128, nch * PIX], BF16, name="tpr", tag="tpr")
            nc.scalar.activation(out=tpr, in_=ps, func=ACT.Identity,
                                 bias=B1R[:, mo, g:g + 1],
                                 scale=A1[:, mo, g:g + 1])
            nc.vector.tensor_scalar(
                out=HP[mo][g][:, c0:c0 + nch, 1:15, 2:16],
                in0=tpr.rearrange("p (s y x) -> p s y x", s=nch, y=14),
                scalar1=128.0, scalar2=XB[:, g:g + 1],
                op0=ALU.max, op1=ALU.min)

        def conv1(g, pool_rr=True):
            for mo in range(2):
                for ch in chunks[g]:
                    nch = len(ch)
                    ps = mmp.tile([128, nch * PIX], F32, name="c1ps", tag="mm")
                    for kt in range(8):
                        nc.tensor.matmul(
                            ps,
                            W1[g][:, kt, mo * 128:(mo + 1) * 128],
                            XQ[:, kt, ch[0] * PIX:(ch[0] + nch) * PIX],
                            start=(kt == 0), stop=(kt == 7))
                    c1_post(g, mo, ch, ps, pool_rr)

        def c2_post(g, mo, ch, ps, pool_rr=None):
            nch = len(ch)
            c0 = ch[0] - slot0[g]
            tpr = rot.tile([128, nch * PIX], BF16, name="tpr", tag="tpr")
            nc.scalar.activation(
                out=tpr, in_=ps.rearrange("p s y x -> p (s y x)"),
                func=ACT.Identity,
                bias=B2R[:, mo, g:g + 1], scale=A2[:, mo, g:g + 1])
            nc.vector.tensor_scalar(
                out=Q2[mo][g][:, c0 * PIX:(c0 + nch) * PIX],
                in0=tpr, scalar1=128.0, scalar2=XB[:, g:g + 1],
                op0=ALU.max, op1=ALU.min)

        def conv2(g, pool_rr=True, defer_posts=False, mos=(0, 1)):
            posts = []
            for mo in mos:
                for ch in chunks[g]:
                    nch = len(ch)
                    c0 = ch[0] - slot0[g]
                    ps = mmp.tile([128, nch, 14, 14], F32, name="c2ps",
                                  tag="mm")
                    first = True
                    for ti, (dy, dx) in enumerate(
                            (dy, dx) for dy in range(3) for dx in range(3)):
                        for kt in range(2):
                            nc.tensor.matmul(
                                ps,
                                W2[g][:, ti, kt, mo * 128:(mo + 1) * 128],
                                HP[kt][g][:, c0:c0 + nch,
                                          dy:dy + 14, dx + 1:dx + 15],
                                start=first, stop=(ti == 8 and kt == 1))
                            first = False
                    if defer_posts:
                        posts.append((mo, ch, ps))
                    else:
                        c2_post(g, mo, ch, ps, pool_rr)
            return posts

        def conv3(g):
            """psum -> ACT drain to S3 sbuf bf16 (removing the 128-offset
            contribution via the -D3 bias); bn_stats reads S3."""
            ns = group_sizes[g]
            bstv = BST[g][:, 0:8 * ns * 6].rearrange("p (t c) -> p t c", c=6)
            for mo in range(8):
                for ch in chunks[g]:
                    nch = len(ch)
                    c0 = ch[0] - slot0[g]
                    ps = mmp.tile([128, nch * PIX], F32, name="c3ps", tag="mm")
                    for kt in range(2):
                        nc.tensor.matmul(
                            ps,
                            W3[g][:, kt, mo * 128:(mo + 1) * 128],
                            Q2[kt][g][:, c0 * PIX:(c0 + nch) * PIX],
                            start=(kt == 0), stop=(kt == 1))
                    nc.scalar.activation(
                        out=S3[g][:, mo, c0 * PIX:(c0 + nch) * PIX],
                        in_=ps, func=ACT.Identity,
                        bias=D3[:, mo, g:g + 1], scale=1.0)
                for si in range(ns):
                    nc.vector.bn_stats(
                        out=bstv[:, mo * ns + si:mo * ns + si + 1, :],
                        in_=S3[g][:, mo, si * PIX:(si + 1) * PIX])

        def stats(g):
            ns = group_sizes[g]
            nst = 8 * ns
            # mean^2 columns (cols 1 and 4 of each 6-tuple)
            mvi = BST[g][:, 0:nst * 6].rearrange(
                "p (t h c) -> p t h c", h=2, c=3)[:, :, :, 1]
            msq = BST[g][:, nst * 6:nst * 8].rearrange("p (t h) -> p t h", h=2)
            nc.vector.tensor_tensor(out=msq, in0=mvi, in1=mvi, op=ALU.mult)
            # partition reduce
            red = smp.tile([1, nst * 8], F32, name="red", tag="red")
            nc.tensor.matmul(red, ONES, BST[g], start=True, stop=True)
            Tg = res.tile([1, nst * 8], F32, name=f"T{g}", tag=f"T{g}")
            nc.scalar.activation(out=Tg, in_=red, func=ACT.Copy,
                                 bias=0.0, scale=1.0)
            return Tg

        def chain(g, Tg):
            """mo-parity pair-add + scalar math -> Fv [1, 8*ns]."""
            ns = group_sizes[g]
            nst = 8 * ns
            nsc = 4 * ns
            TB = res.tile([1, 4 * ns * 8], F32, name=f"TB{g}", tag=f"TB{g}")
            tv = Tg[:, 0:nst * 6].rearrange("p (m o s c) -> p m o s c",
                                            m=4, o=2, c=6)
            nc.vector.tensor_tensor(
                out=TB[:, 0:4 * ns * 6].rearrange("p (m s c) -> p m s c",
                                                  m=4, c=6),
                in0=tv[:, :, 0, :, :], in1=tv[:, :, 1, :, :], op=ALU.add)
            mv = Tg[:, nst * 6:nst * 8].rearrange("p (m o s c) -> p m o s c",
                                                  m=4, o=2, c=2)
            nc.vector.tensor_tensor(
                out=TB[:, 4 * ns * 6:4 * ns * 8].rearrange(
                    "p (m s c) -> p m s c", m=4, c=2),
                in0=mv[:, :, 0, :, :], in1=mv[:, :, 1, :, :], op=ALU.add)
            tb6 = TB[:, 0:4 * ns * 6].rearrange("p (t c) -> p t c", c=6)
            tb2 = TB[:, 4 * ns * 6:4 * ns * 8].rearrange("p (t c) -> p t c",
                                                         c=2)
            SC = res.tile([1, nsc * 4], F32, name=f"SC{g}", tag=f"SC{g}")
            scv = SC.rearrange("p (c t) -> p c t", c=4)
            nc.vector.tensor_tensor(out=scv[:, 0, :], in0=tb6[:, :, 1],
                                    in1=tb6[:, :, 4], op=ALU.add)
            nc.vector.tensor_tensor(out=scv[:, 1, :], in0=tb6[:, :, 2],
                                    in1=tb6[:, :, 5], op=ALU.add)
            nc.vector.tensor_tensor(out=scv[:, 2, :], in0=tb2[:, :, 0],
                                    in1=tb2[:, :, 1], op=ALU.add)
            MEAN = rot.tile([1, nsc], F32, name="MEAN", tag=f"MEAN{g}")
            nc.vector.tensor_scalar(out=MEAN, in0=scv[:, 0, :],
                                    scalar1=1.0 / 512, scalar2=None,
                                    op0=ALU.mult)
            E2 = rot.tile([1, nsc], F32, name="E2", tag=f"E2{g}")
            nc.vector.scalar_tensor_tensor(out=E2, in0=scv[:, 2, :],
                                           scalar=98.0, in1=scv[:, 1, :],
                                           op0=ALU.mult, op1=ALU.add)
            nc.vector.tensor_scalar(out=E2, in0=E2,
                                    scalar1=1.0 / (2 * 128 * PIX),
                                    scalar2=None, op0=ALU.mult)
            VAR = rot.tile([1, nsc], F32, name="VAR", tag=f"VAR{g}")
            nc.vector.tensor_tensor(out=VAR, in0=MEAN, in1=MEAN, op=ALU.mult)
            nc.vector.tensor_tensor(out=VAR, in0=E2, in1=VAR, op=ALU.subtract)
            cbase = 1024 + sum(16 * n for n in group_sizes[:g])
            nc.vector.tensor_tensor(out=VAR, in0=VAR,
                                    in1=GR[:, cbase + nsc:cbase + 2 * nsc],
                                    op=ALU.mult)
            nc.vector.tensor_scalar(out=VAR, in0=VAR, scalar1=EPS,
                                    scalar2=None, op0=ALU.add)
            SD = rot.tile([1, nsc], F32, name="SD", tag=f"SD{g}")
            nc.scalar.activation(out=SD, in_=VAR, func=ACT.Sqrt,
                                 bias=0.0, scale=1.0)
            RC = rot.tile([1, nsc], F32, name="RC", tag=f"RC{g}")
            nc.vector.reciprocal(out=RC, in_=SD)
            Fv = res.tile([1, 8 * ns], F32, name=f"F_{g}", tag=f"F_{g}")
            nc.vector.tensor_tensor(out=Fv[:, 0:nsc], in0=RC,
                                    in1=GR[:, cbase:cbase + nsc], op=ALU.mult)
            nc.vector.scalar_tensor_tensor(
                out=Fv[:, nsc:2 * nsc], in0=MEAN, scalar=-1.0,
                in1=Fv[:, 0:nsc], op0=ALU.mult, op1=ALU.mult)
            return Fv

        def pq_outer(g, Fv):
            """P,Q outer products on PE; ACT drain."""
            ns = group_sizes[g]
            pqp = smp.tile([128, 8, 2, ns], F32, name="pqp", tag="pqp")
            fvv = Fv.rearrange("p (k m s) -> p k m s", k=2, m=4)
            for mo in range(8):
                nc.tensor.matmul(
                    pqp[:, mo, :, :],
                    GNG[:, mo * 128:(mo + 1) * 128],
                    fvv[:, :, mo // 2, :],
                    start=(mo == 0), stop=(mo == 7), skip_group_check=True)
            PQ[g] = res.tile([128, 8, 2, ns], F32, name=f"PQ{g}", tag=f"PQ{g}")
            nc.scalar.activation(out=PQ[g], in_=pqp, func=ACT.Copy,
                                 bias=0.0, scale=1.0)

        def qg_make(g):
            ns = group_sizes[g]
            cb = sum(8 * n for n in group_sizes[:g])
            gnbx = CC2[:, cb:cb + 8 * ns].rearrange("p (m s) -> p m s", m=8)
            QG[g] = res.tile([128, 8, ns], F32, name=f"QG{g}", tag=f"QG{g}")
            nc.vector.tensor_tensor(out=QG[g], in0=PQ[g][:, :, 1, :],
                                    in1=gnbx, op=ALU.add)

        VT = [None] * NG

        def apply_affine(g, mos):
            """DVE affine_then_add: V = S3*P + QG + XR per (mo, si)."""
            ns = group_sizes[g]
            if VT[g] is None:
                VT[g] = res.tile([128, 8, ns * PIX], BF16, name=f"VT{g}",
                                 tag=f"VT{g}")
            for mo in mos:
                for si, slot in enumerate(groups[g]):
                    nc.vector.affine_then_add(
                        out=VT[g][:, mo, si * PIX:(si + 1) * PIX],
                        in0=S3[g][:, mo, si * PIX:(si + 1) * PIX],
                        in1=XR[:, mo, slot * PIX:(slot + 1) * PIX],
                        scale=PQ[g][:, mo, 0, si:si + 1],
                        bias=QG[g][:, mo, si:si + 1])

        def apply_relu(g, mos, engine):
            """relu(V) -> OT (one op per mo), DMA out per 4-mo block."""
            ns = group_sizes[g]
            for mo in mos:
                if engine == "dve":
                    nc.vector.tensor_scalar(
                        out=OT[g][:, mo, :], in0=VT[g][:, mo, :],
                        scalar1=0.0, scalar2=None, op0=ALU.max)
                else:
                    nc.scalar.activation(
                        out=OT[g][:, mo, :], in_=VT[g][:, mo, :],
                        func=ACT.Relu, bias=0.0, scale=1.0)
                nc.sync.dma_start(
                    out=out_d.ap()[:, mo,
                                   slot0[g] * PIX:(slot0[g] + ns) * PIX],
                    in_=OT[g][:, mo, :])

        def apply_last(g):
            """Last group (ns==1): DVE affine_then_add from S3 + ACT relu."""
            ns = group_sizes[g]
            slot = groups[g][0]
            for mo in range(8):
                V = rot.tile([128, ns * PIX], BF16, name="V", tag="Vl")
                nc.vector.affine_then_add(
                    out=V,
                    in0=S3[g][:, mo, :],
                    in1=XR[:, mo, slot * PIX:(slot + 1) * PIX],
                    scale=PQ[g][:, mo, 0, 0:1],
                    bias=QG[g][:, mo, 0:1])
                nc.scalar.activation(
                    out=OT[g][:, mo, :], in_=V, func=ACT.Relu,
                    bias=0.0, scale=1.0)
                if mo in (3, 7):
                    nc.sync.dma_start(
                        out=out_d.ap()[:, mo - 3:mo + 1,
                                       slot0[g] * PIX:(slot0[g] + ns) * PIX],
                        in_=OT[g][:, mo - 3:mo + 1, :])

        # ---------------- schedule ----------------
        gl = NG - 1
        if NG == 2:
            conv1(0)
            conv2(0)
            conv3(0)
            Tg0 = stats(0)
            conv1(gl)
            Fv0 = chain(0, Tg0)
            conv2(gl, mos=(0,))
            pq_outer(0, Fv0)
            qg_make(0)
            conv2(gl, mos=(1,))
            apply_affine(0, range(0, 4))
            apply_relu(0, range(0, 4), "act")
            conv3(gl)
            Tg1 = stats(gl)
            Fv1 = chain(gl, Tg1)
            pq_outer(gl, Fv1)
            qg_make(gl)
            apply_last(gl)
            apply_affine(0, range(4, 8))
            apply_relu(0, range(4, 8), "act")
        else:
            # generic fallback (e.g. (2,2) grouping)
            c1_done = set()
            for g in range(NG):
                if g not in c1_done:
                    conv1(g)
                    c1_done.add(g)
                conv2(g)
                conv3(g)
                Tg = stats(g)
                if g == NG - 2:
                    conv1(gl)
                    c1_done.add(gl)
                Fv = chain(g, Tg)
                pq_outer(g, Fv)
                qg_make(g)
                if g == NG - 1 and group_sizes[g] == 1:
                    apply_last(g)
                else:
                    apply_affine(g, range(0, 8))
                    apply_relu(g, range(0, 8), "dve")

    nc.compile()
    return nc


# ----------------------------------------------------------------------------
# Host side
# ----------------------------------------------------------------------------

def _quant_w(w, lv):
    n = max(lv // 2 - 1, 1)
    s = np.float32(np.abs(w).max()) + np.float32(1e-12)
    k = np.round((w.astype(np.float32) / s) * np.float32(n)).astype(np.float32)
    return k, np.float32(s) / np.float32(n)


def _assign_groups(mask):
    mask = np.asarray(mask).astype(np.int64)
    ids = {e: [int(i) for i in np.nonzero(mask == e)[0]] for e in range(3)}
    counts = [len(ids[e]) for e in range(3)]
    if all(c % 2 == 0 for c in counts):
        group_sizes = (2, 2)
        chunks2 = []
        for e in range(3):
            for j in range(0, counts[e], 2):
                chunks2.append((e, ids[e][j:j + 2]))
        assert len(chunks2) == 16
        core_samples = []
        core_experts = []
        for c in range(8):
            (ea, sa), (eb, sb) = chunks2[2 * c], chunks2[2 * c + 1]
            core_samples.append(sa + sb)
            core_experts.append([ea, eb])
        return group_sizes, core_samples, core_experts

    base = [c % 3 for c in counts]
    need = (8 - sum(base)) // 3
    t = [0, 0, 0]
    for e in range(3):
        cap = (counts[e] - base[e]) // 3
        take = min(cap, need)
        t[e] = take
        need -= take
        if need == 0:
            break
    assert need == 0
    b = [base[e] + 3 * t[e] for e in range(3)]
    a = [(counts[e] - b[e]) // 3 for e in range(3)]
    assert sum(a) == 8 and sum(b) == 8
    trip = []
    single = []
    for e in range(3):
        pos = 0
        for _ in range(a[e]):
            trip.append((e, ids[e][pos:pos + 3]))
            pos += 3
        for _ in range(b[e]):
            single.append((e, [ids[e][pos]]))
            pos += 1
        assert pos == counts[e]
    core_samples = []
    core_experts = []
    for c in range(8):
        ea, sa = trip[c]
        eb, sb = single[c]
        core_samples.append(sa + sb)
        core_experts.append([ea, eb])
    return (3, 1), core_samples, core_experts


def kernel(x, mask, w1, w2, w3, bn1_g, bn1_b, bn1_m, bn1_v,
           bn2_g, bn2_b, bn2_m, bn2_v, gn_g, gn_b):
    import ml_dtypes
    from concourse.bass_utils import run_bass_kernel_spmd

    bf16 = ml_dtypes.bfloat16
    f32 = np.float32
    x = np.asarray(x, f32)
    mask = np.asarray(mask)
    w1 = np.asarray(w1, f32)
    w2 = np.asarray(w2, f32)
    w3 = np.asarray(w3, f32)
    bn1 = [np.asarray(v, f32) for v in (bn1_g, bn1_b, bn1_m, bn1_v)]
    bn2 = [np.asarray(v, f32) for v in (bn2_g, bn2_b, bn2_m, bn2_v)]
    gn_g = np.asarray(gn_g, f32)
    gn_b = np.asarray(gn_b, f32)

    group_sizes, core_samples, core_experts = _assign_groups(mask)
    NG = len(group_sizes)

    lv_of = [2 ** b for b in BITS]
    K1, K2, K3 = {}, {}, {}
    CW = {}
    for e in range(3):
        lv = lv_of[e]
        k1, c1 = _quant_w(w1, lv)
        k2, c2 = _quant_w(w2, lv)
        k3, c3 = _quant_w(w3, lv)
        K1[e] = k1.reshape(256, 1024)
        K2[e] = k2.reshape(256, 256, 3, 3)
        K3[e] = k3.reshape(1024, 256)
        CW[e] = (c1, c2, c3)

    inv1 = bn1[0] / np.sqrt(bn1[3] + f32(EPS))
    bb1 = bn1[1] - bn1[2] * inv1
    inv2 = bn2[0] / np.sqrt(bn2[3] + f32(EPS))
    bb2 = bn2[1] - bn2[2] * inv2

    def pack_w(e):
        k1t = K1[e].T.reshape(8, 128, 256).transpose(1, 0, 2)
        k2t = K2[e].transpose(2, 3, 1, 0).reshape(9, 2, 128, 256)
        k2t = k2t.transpose(2, 0, 1, 3)
        k3t = K3[e].T.reshape(2, 128, 1024).transpose(1, 0, 2)
        return (np.ascontiguousarray(k1t).astype(bf16),
                np.ascontiguousarray(k2t).astype(bf16),
                np.ascontiguousarray(k3t).astype(bf16))

    packed = {e: pack_w(e) for e in set(int(v) for v in np.asarray(mask))}

    # host-side input quantization per sample (exact integer grid)
    lv_smp = np.array([lv_of[int(mask[s])] for s in range(B)], f32)
    xq_full = np.clip(np.round(x * (lv_smp - 1)[:, None, None, None]),
                      0.0, (lv_smp - 1)[:, None, None, None]).astype(f32)

    in_maps = []
    for c in range(8):
        sids = core_samples[c]
        experts = core_experts[c]

        # [128, 8, 4*196] channel-tile major
        xqc = xq_full[sids].reshape(4, 8, 128, PIX).transpose(2, 1, 0, 3) \
                           .reshape(128, 8, 4 * PIX)
        xrc = x[sids].reshape(4, 8, 128, PIX).transpose(2, 1, 0, 3) \
                     .reshape(128, 8, 4 * PIX)

        w1c = np.stack([packed[experts[g]][0] for g in range(NG)])
        w2c = np.stack([packed[experts[g]][1] for g in range(NG)])
        w3c = np.stack([packed[experts[g]][2] for g in range(NG)])

        glv = [lv_of[experts[g]] for g in range(NG)]
        NCC = 4 * (2 * NG) + NG + 8 + 8 * NG
        cc = np.zeros((128, NCC), f32)
        a1 = np.zeros((128, 2, NG), f32)
        b1 = np.zeros((128, 2, NG), f32)
        a2 = np.zeros((128, 2, NG), f32)
        b2 = np.zeros((128, 2, NG), f32)
        d3 = np.zeros((128, 8, NG), f32)
        for g in range(NG):
            e = experts[g]
            lv = glv[g]
            c1, c2, c3 = CW[e]
            # offset-128 storage: +128 into the quantizing biases; the
            # 128*rowsum(w) contribution of the offset inputs is removed from
            # the next stage (conv2 bias) or the S3 drain bias (conv3).
            w2sum = K2[e].sum(axis=(1, 2, 3))          # (256,)
            w3sum = K3[e].sum(axis=1)                  # (1024,)
            a1[:, :, g] = (inv1 * c1).reshape(2, 128).T
            b1[:, :, g] = (bb1 * f32(lv - 1)).reshape(2, 128).T + f32(128.0)
            a2[:, :, g] = (inv2 * c2).reshape(2, 128).T
            b2[:, :, g] = (bb2 * f32(lv - 1)
                           - inv2 * c2 * f32(128.0) * w2sum
                           ).reshape(2, 128).T + f32(128.0)
            d3[:, :, g] = (-f32(128.0) * w3sum).reshape(8, 128).T
        o = 0
        cc[:, o:o + 2 * NG] = a1.reshape(128, 2 * NG); o += 2 * NG
        cc[:, o:o + 2 * NG] = b1.reshape(128, 2 * NG); o += 2 * NG
        cc[:, o:o + 2 * NG] = a2.reshape(128, 2 * NG); o += 2 * NG
        cc[:, o:o + 2 * NG] = b2.reshape(128, 2 * NG); o += 2 * NG
        cc[:, o:o + NG] = [128.0 + lv - 1 for lv in glv]; o += NG
        cc[:, o:o + 8] = gn_b.reshape(8, 128).T; o += 8
        cc[:, o:o + 8 * NG] = d3.reshape(128, 8 * NG); o += 8 * NG

        NGR = 1024 + sum(16 * n for n in group_sizes)
        gr = np.zeros((1, NGR), f32)
        gr[0, 0:1024] = gn_g
        off = 1024
        for g in range(NG):
            ns = group_sizes[g]
            e = experts[g]
            lv = glv[g]
            c3e = CW[e][2] / f32(lv - 1)
            gr[0, off:off + 4 * ns] = c3e
            gr[0, off + 4 * ns:off + 8 * ns] = c3e * c3e
            off += 16 * ns

        NC2 = sum(8 * n for n in group_sizes)
        cc2 = np.zeros((128, NC2), f32)
        cb = 0
        gnbp = gn_b.reshape(8, 128).T  # [128, 8]
        for g in range(NG):
            ns = group_sizes[g]
            cc2[:, cb:cb + 8 * ns] = np.repeat(gnbp, ns, axis=1)
            cb += 8 * ns

        in_maps.append({
            "xq": xqc.astype(bf16), "xr": xrc.astype(bf16),
            "w1": w1c, "w2": w2c, "w3": w3c,
            "cc": cc, "gr": gr, "cc2": cc2,
        })

    key = group_sizes
    if key not in _NC_CACHE:
        _NC_CACHE[key] = _build_nc(group_sizes)
    nc = _NC_CACHE[key]

    res = run_bass_kernel_spmd(nc, in_maps, core_ids=list(range(NCORES)))

    out = np.zeros((B, OUTC, H, W), f32)
    for c in range(8):
        oc = np.asarray(res.results[c]["out"], dtype=f32)  # [128, 8, 4*PIX]
        oc = oc.reshape(128, 8, 4, PIX).transpose(2, 1, 0, 3) \
               .reshape(4, OUTC, H, W)
        for t, sid in enumerate(core_samples[c]):
            out[sid] = oc[t]
    return out


# revision 31
# speedup vs baseline: 1.1591x; 1.0056x over previous
"""Trainium2 Bass kernel for quantized-MoE Bottleneck (nn_Bottleneck_37503654429269).

v6 layout:
- Host precomputes quantized activations Xq (bf16 integers) and ships the
  residual x as bf16; no device-side input quantization.
- Offset-128 storage for intermediate quantized activations: the bn affine
  is written by ACT directly as bf16 with +128 folded into the bias, so the
  bf16 output rounding IS the integer rounding; one DVE clamp
  (max 128, min 128+XB) finishes the quantization. Host folds the
  128*rowsum(w) corrections into the next stage's bias (conv2) or the S3
  drain bias (conv3).
- Group-major schedule: g0 conv1->2->3 completes early; its GN apply
  overlaps g1's convs. All conv3 outputs drain to S3 sbuf (bf16) with the
  offset correction applied; bn_stats reads S3 so PSUM recycles fast.
- DMA: priority-ordered on the SP queue (W1/XQ of g0 first); late tensors
  (XR, g1 conv2/conv3 weights) issue from the idle GpSimd queue.
"""

import numpy as np

BITS = (2, 4, 8)
EPS = 1e-5
B, C_IN, H, W = 32, 1024, 14, 14
WIDTH, OUTC = 256, 1024
PIX = H * W  # 196
NCORES = 8
RB = float(2.0 ** 23)

_NC_CACHE = {}


# ----------------------------------------------------------------------------
# Device program
# ----------------------------------------------------------------------------

def _build_nc(group_sizes):
    from contextlib import ExitStack
    import concourse.bacc as bacc
    import concourse.mybir as mybir
    import concourse.tile as tile

    F32 = mybir.dt.float32
    BF16 = mybir.dt.bfloat16
    ALU = mybir.AluOpType
    ACT = mybir.ActivationFunctionType

    NG = len(group_sizes)
    NS = sum(group_sizes)
    assert NS == 4
    slot0 = [sum(group_sizes[:g]) for g in range(NG)]
    groups = [list(range(slot0[g], slot0[g] + group_sizes[g])) for g in range(NG)]
    chunks = {g: [groups[g][i:i + 2] for i in range(0, len(groups[g]), 2)]
              for g in range(NG)}
    # last group should be the smallest (shortest tail)
    NSL = group_sizes[-1]

    nc = bacc.Bacc("TRN2", target_bir_lowering=False, debug=False,
                   num_devices=NCORES)

    # ---- dram tensors
    # xq: quantized activations [128, kt(8), 4*196] bf16 (integers)
    xq_d = nc.dram_tensor("xq", [128, 8, 4 * PIX], BF16, kind="ExternalInput")
    # xr: residual x [128, mo(8), 4*196] bf16
    xr_d = nc.dram_tensor("xr", [128, 8, 4 * PIX], BF16, kind="ExternalInput")
    w1_d = nc.dram_tensor("w1", [NG, 128, 8, 256], BF16, kind="ExternalInput")
    w2_d = nc.dram_tensor("w2", [NG, 128, 9, 2, 256], BF16, kind="ExternalInput")
    w3_d = nc.dram_tensor("w3", [NG, 128, 2, 1024], BF16, kind="ExternalInput")
    # packed per-partition consts:
    # a1[2,NG] b1r[2,NG] a2[2,NG] b2r[2,NG] xb[NG] gnb[8] d3[8,NG]
    NCC = 4 * (2 * NG) + NG + 8 + 8 * NG
    cc_d = nc.dram_tensor("cc", [128, NCC], F32, kind="ExternalInput")
    # row consts: gng[1024] + per-g (c3e[4*ns], c3e2[4*ns]) + gnbx[8*ns per g]
    NGR = 1024 + sum(16 * n for n in group_sizes)
    gr_d = nc.dram_tensor("gr", [1, NGR], F32, kind="ExternalInput")
    out_d = nc.dram_tensor("out", [128, 8, 4 * PIX], BF16, kind="ExternalOutput")

    with tile.TileContext(nc) as tc, ExitStack() as ctx:
        res = ctx.enter_context(tc.tile_pool(name="res", bufs=1))
        rot = ctx.enter_context(tc.tile_pool(name="rot", bufs=4))
        mmp = ctx.enter_context(tc.tile_pool(name="mmp", bufs=5, space="PSUM"))
        smp = ctx.enter_context(tc.tile_pool(name="smp", bufs=1, space="PSUM"))

        # ---- persistent tiles
        CC = res.tile([128, NCC], F32, name="CC", tag="CC")
        o = 0
        A1 = CC[:, o:o + 2 * NG].rearrange("p (m g) -> p m g", m=2); o += 2 * NG
        B1R = CC[:, o:o + 2 * NG].rearrange("p (m g) -> p m g", m=2); o += 2 * NG
        A2 = CC[:, o:o + 2 * NG].rearrange("p (m g) -> p m g", m=2); o += 2 * NG
        B2R = CC[:, o:o + 2 * NG].rearrange("p (m g) -> p m g", m=2); o += 2 * NG
        XB = CC[:, o:o + NG]; o += NG
        GNB = CC[:, o:o + 8]; o += 8
        D3 = CC[:, o:o + 8 * NG].rearrange("p (m g) -> p m g", m=8); o += 8 * NG

        GR = res.tile([1, NGR], F32, name="GR", tag="GR")
        GNG = GR[:, 0:1024]

        # gnbx: per-partition gn_b replicated per sample, [128, 8*ns] per group
        NC2 = sum(8 * n for n in group_sizes)
        cc2_d = nc.dram_tensor("cc2", [128, NC2], F32, kind="ExternalInput")
        CC2 = res.tile([128, NC2], F32, name="CC2", tag="CC2")

        XQ = res.tile([128, 8, 4 * PIX], BF16, name="XQ", tag="XQ")
        XR = res.tile([128, 8, 4 * PIX], BF16, name="XR", tag="XR")
        W1 = [res.tile([128, 8, 256], BF16, name=f"W1_{g}", tag=f"W1_{g}")
              for g in range(NG)]
        W2 = [res.tile([128, 9, 2, 256], BF16, name=f"W2_{g}", tag=f"W2_{g}")
              for g in range(NG)]
        W3 = [res.tile([128, 2, 1024], BF16, name=f"W3_{g}", tag=f"W3_{g}")
              for g in range(NG)]

        # ---- DMA: single SP queue in strict need-order; XQ-g0 split per
        # kt-pair so conv1's psum accumulation can start on the first pair.
        n0 = group_sizes[0] * PIX
        # first-needed tensors issue from the scalar/vector queues, which are
        # idle at kernel start: their transfers begin ~2us before SP's.
        nc.scalar.dma_start(out=W1[0], in_=w1_d.ap()[0])
        nc.scalar.dma_start(out=XQ[:, 0:2, 0:n0],
                            in_=xq_d.ap()[:, 0:2, 0:n0])
        nc.gpsimd.dma_start(out=XQ[:, 2:4, 0:n0],
                            in_=xq_d.ap()[:, 2:4, 0:n0])
        nc.sync.dma_start(out=CC, in_=cc_d.ap())
        nc.sync.dma_start(out=CC2, in_=cc2_d.ap())
        nc.sync.dma_start(out=GR, in_=gr_d.ap())
        for kp in range(2, 4):
            nc.sync.dma_start(out=XQ[:, 2 * kp:2 * kp + 2, 0:n0],
                              in_=xq_d.ap()[:, 2 * kp:2 * kp + 2, 0:n0])
        nc.sync.dma_start(out=W2[0], in_=w2_d.ap()[0])
        nc.sync.dma_start(out=W3[0], in_=w3_d.ap()[0])
        for g in range(1, NG):
            nc.sync.dma_start(
                out=XQ[:, :, slot0[g] * PIX:(slot0[g] + group_sizes[g]) * PIX],
                in_=xq_d.ap()[:, :, slot0[g] * PIX:(slot0[g] + group_sizes[g]) * PIX])
            nc.sync.dma_start(out=W1[g], in_=w1_d.ap()[g])
        nc.sync.dma_start(out=XR, in_=xr_d.ap())
        for g in range(1, NG):
            nc.sync.dma_start(out=W2[g], in_=w2_d.ap()[g])
            nc.sync.dma_start(out=W3[g], in_=w3_d.ap()[g])

        ONES = res.tile([128, 1], F32, name="ONES", tag="ONES")
        nc.vector.memset(ONES, 1.0)

        # HP padded conv2 inputs, zero ring (Pool memsets, early)
        HP = [[res.tile([128, group_sizes[g], 16, 18], BF16,
                        name=f"HP{kt}_{g}", tag=f"HP{kt}_{g}")
               for g in range(NG)] for kt in range(2)]
        for kt in range(2):
            for g in range(NG):
                nc.gpsimd.memset(HP[kt][g], 128.0)

        Q2 = [[res.tile([128, group_sizes[g] * PIX], BF16,
                        name=f"Q2{kt}_{g}", tag=f"Q2{kt}_{g}")
               for g in range(NG)] for kt in range(2)]
        # S3 sbuf (bf16, offset-corrected conv3 output) for all groups
        S3 = [res.tile([128, 8, group_sizes[g] * PIX], BF16,
                       name=f"S3_{g}", tag=f"S3_{g}")
              for g in range(NG)]

        BST = [res.tile([128, 8 * group_sizes[g] * 8], F32, name=f"BST{g}",
                        tag=f"BST{g}") for g in range(NG)]
        PQ = [None] * NG
        QG = [None] * NG
        OT = [res.tile([128, 8, group_sizes[g] * PIX], BF16,
                       name=f"OT{g}", tag=f"OT{g}") for g in range(NG)]

        def c1_post(g, mo, ch, ps, pool_rr=None):
            nch = len(ch)
            c0 = ch[0] - slot0[g]
            # bf16 store of a*ps + b + 128 rounds to the integer grid in
            # [128, 256); one clamp finishes quantization (offset-128 kept).
            tpr = rot.tile([128, nch * PIX], BF16, name="tpr", tag="tpr")
            nc.scalar.activation(out=tpr, in_=ps, func=ACT.Identity,
                                 bias=B1R[:, mo, g:g + 1],
                                 scale=A1[:, mo, g:g + 1])
            nc.vector.tensor_scalar(
                out=HP[mo][g][:, c0:c0 + nch, 1:15, 2:16],
                in0=tpr.rearrange("p (s y x) -> p s y x", s=nch, y=14),
                scalar1=128.0, scalar2=XB[:, g:g + 1],
                op0=ALU.max, op1=ALU.min)

        def conv1(g, pool_rr=True):
            for mo in range(2):
                for ch in chunks[g]:
                    nch = len(ch)
                    ps = mmp.tile([128, nch * PIX], F32, name="c1ps", tag="mm")
                    for kt in range(8):
                        nc.tensor.matmul(
                            ps,
                            W1[g][:, kt, mo * 128:(mo + 1) * 128],
                            XQ[:, kt, ch[0] * PIX:(ch[0] + nch) * PIX],
                            start=(kt == 0), stop=(kt == 7))
                    c1_post(g, mo, ch, ps, pool_rr)

        def c2_post(g, mo, ch, ps, pool_rr=None):
            nch = len(ch)
            c0 = ch[0] - slot0[g]
            tpr = rot.tile([128, nch * PIX], BF16, name="tpr", tag="tpr")
            nc.scalar.activation(
                out=tpr, in_=ps.rearrange("p s y x -> p (s y x)"),
                func=ACT.Identity,
                bias=B2R[:, mo, g:g + 1], scale=A2[:, mo, g:g + 1])
            nc.vector.tensor_scalar(
                out=Q2[mo][g][:, c0 * PIX:(c0 + nch) * PIX],
                in0=tpr, scalar1=128.0, scalar2=XB[:, g:g + 1],
                op0=ALU.max, op1=ALU.min)

        def conv2(g, pool_rr=True, defer_posts=False, mos=(0, 1)):
            posts = []
            for mo in mos:
                for ch in chunks[g]:
                    nch = len(ch)
                    c0 = ch[0] - slot0[g]
                    ps = mmp.tile([128, nch, 14, 14], F32, name="c2ps",
                                  tag="mm")
                    first = True
                    for ti, (dy, dx) in enumerate(
                            (dy, dx) for dy in range(3) for dx in range(3)):
                        for kt in range(2):
                            nc.tensor.matmul(
                                ps,
                                W2[g][:, ti, kt, mo * 128:(mo + 1) * 128],
                                HP[kt][g][:, c0:c0 + nch,
                                          dy:dy + 14, dx + 1:dx + 15],
                                start=first, stop=(ti == 8 and kt == 1))
                            first = False
                    if defer_posts:
                        posts.append((mo, ch, ps))
                    else:
                        c2_post(g, mo, ch, ps, pool_rr)
            return posts

        def conv3(g):
            """psum -> ACT drain to S3 sbuf bf16 (removing the 128-offset
            contribution via the -D3 bias); bn_stats reads S3."""
            ns = group_sizes[g]
            bstv = BST[g][:, 0:8 * ns * 6].rearrange("p (t c) -> p t c", c=6)
            for mo in range(8):
                for ch in chunks[g]:
                    nch = len(ch)
                    c0 = ch[0] - slot0[g]
                    ps = mmp.tile([128, nch * PIX], F32, name="c3ps", tag="mm")
                    for kt in range(2):
                        nc.tensor.matmul(
                            ps,
                            W3[g][:, kt, mo * 128:(mo + 1) * 128],
                            Q2[kt][g][:, c0 * PIX:(c0 + nch) * PIX],
                            start=(kt == 0), stop=(kt == 1))
                    nc.scalar.activation(
                        out=S3[g][:, mo, c0 * PIX:(c0 + nch) * PIX],
                        in_=ps, func=ACT.Identity,
                        bias=D3[:, mo, g:g + 1], scale=1.0)
                for si in range(ns):
                    nc.vector.bn_stats(
                        out=bstv[:, mo * ns + si:mo * ns + si + 1, :],
                        in_=S3[g][:, mo, si * PIX:(si + 1) * PIX])

        def stats(g):
            ns = group_sizes[g]
            nst = 8 * ns
            # mean^2 columns (cols 1 and 4 of each 6-tuple)
            mvi = BST[g][:, 0:nst * 6].rearrange(
                "p (t h c) -> p t h c", h=2, c=3)[:, :, :, 1]
            msq = BST[g][:, nst * 6:nst * 8].rearrange("p (t h) -> p t h", h=2)
            nc.vector.tensor_tensor(out=msq, in0=mvi, in1=mvi, op=ALU.mult)
            # partition reduce
            red = smp.tile([1, nst * 8], F32, name="red", tag="red")
            nc.tensor.matmul(red, ONES, BST[g], start=True, stop=True)
            Tg = res.tile([1, nst * 8], F32, name=f"T{g}", tag=f"T{g}")
            nc.scalar.activation(out=Tg, in_=red, func=ACT.Copy,
                                 bias=0.0, scale=1.0)
            return Tg

        def chain(g, Tg):
            """mo-parity pair-add + scalar math -> Fv [1, 8*ns]."""
            ns = group_sizes[g]
            nst = 8 * ns
            nsc = 4 * ns
            TB = res.tile([1, 4 * ns * 8], F32, name=f"TB{g}", tag=f"TB{g}")
            tv = Tg[:, 0:nst * 6].rearrange("p (m o s c) -> p m o s c",
                                            m=4, o=2, c=6)
            nc.vector.tensor_tensor(
                out=TB[:, 0:4 * ns * 6].rearrange("p (m s c) -> p m s c",
                                                  m=4, c=6),
                in0=tv[:, :, 0, :, :], in1=tv[:, :, 1, :, :], op=ALU.add)
            mv = Tg[:, nst * 6:nst * 8].rearrange("p (m o s c) -> p m o s c",
                                                  m=4, o=2, c=2)
            nc.vector.tensor_tensor(
                out=TB[:, 4 * ns * 6:4 * ns * 8].rearrange(
                    "p (m s c) -> p m s c", m=4, c=2),
                in0=mv[:, :, 0, :, :], in1=mv[:, :, 1, :, :], op=ALU.add)
            tb6 = TB[:, 0:4 * ns * 6].rearrange("p (t c) -> p t c", c=6)
            tb2 = TB[:, 4 * ns * 6:4 * ns * 8].rearrange("p (t c) -> p t c",
                                                         c=2)
            SC = res.tile([1, nsc * 4], F32, name=f"SC{g}", tag=f"SC{g}")
            scv = SC.rearrange("p (c t) -> p c t", c=4)
            nc.vector.tensor_tensor(out=scv[:, 0, :], in0=tb6[:, :, 1],
                                    in1=tb6[:, :, 4], op=ALU.add)
            nc.vector.tensor_tensor(out=scv[:, 1, :], in0=tb6[:, :, 2],
                                    in1=tb6[:, :, 5], op=ALU.add)
            nc.vector.tensor_tensor(out=scv[:, 2, :], in0=tb2[:, :, 0],
                                    in1=tb2[:, :, 1], op=ALU.add)
            MEAN = rot.tile([1, nsc], F32, name="MEAN", tag=f"MEAN{g}")
            nc.vector.tensor_scalar(out=MEAN, in0=scv[:, 0, :],
                                    scalar1=1.0 / 512, scalar2=None,
                                    op0=ALU.mult)
            E2 = rot.tile([1, nsc], F32, name="E2", tag=f"E2{g}")
            nc.vector.scalar_tensor_tensor(out=E2, in0=scv[:, 2, :],
                                           scalar=98.0, in1=scv[:, 1, :],
                                           op0=ALU.mult, op1=ALU.add)
            nc.vector.tensor_scalar(out=E2, in0=E2,
                                    scalar1=1.0 / (2 * 128 * PIX),
                                    scalar2=None, op0=ALU.mult)
            VAR = rot.tile([1, nsc], F32, name="VAR", tag=f"VAR{g}")
            nc.vector.tensor_tensor(out=VAR, in0=MEAN, in1=MEAN, op=ALU.mult)
            nc.vector.tensor_tensor(out=VAR, in0=E2, in1=VAR, op=ALU.subtract)
            cbase = 1024 + sum(16 * n for n in group_sizes[:g])
            nc.vector.tensor_tensor(out=VAR, in0=VAR,
                                    in1=GR[:, cbase + nsc:cbase + 2 * nsc],
                                    op=ALU.mult)
            nc.vector.tensor_scalar(out=VAR, in0=VAR, scalar1=EPS,
                                    scalar2=None, op0=ALU.add)
            SD = rot.tile([1, nsc], F32, name="SD", tag=f"SD{g}")
            nc.scalar.activation(out=SD, in_=VAR, func=ACT.Sqrt,
                                 bias=0.0, scale=1.0)
            RC = rot.tile([1, nsc], F32, name="RC", tag=f"RC{g}")
            nc.vector.reciprocal(out=RC, in_=SD)
            Fv = res.tile([1, 8 * ns], F32, name=f"F_{g}", tag=f"F_{g}")
            nc.vector.tensor_tensor(out=Fv[:, 0:nsc], in0=RC,
                                    in1=GR[:, cbase:cbase + nsc], op=ALU.mult)
            nc.vector.scalar_tensor_tensor(
                out=Fv[:, nsc:2 * nsc], in0=MEAN, scalar=-1.0,
                in1=Fv[:, 0:nsc], op0=ALU.mult, op1=ALU.mult)
            return Fv

        def pq_outer(g, Fv):
            """P,Q outer products on PE; ACT drain."""
            ns = group_sizes[g]
            pqp = smp.tile([128, 8, 2, ns], F32, name="pqp", tag="pqp")
            fvv = Fv.rearrange("p (k m s) -> p k m s", k=2, m=4)
            for mo in range(8):
                nc.tensor.matmul(
                    pqp[:, mo, :, :],
                    GNG[:, mo * 128:(mo + 1) * 128],
                    fvv[:, :, mo // 2, :],
                    start=(mo == 0), stop=(mo == 7), skip_group_check=True)
            PQ[g] = res.tile([128, 8, 2, ns], F32, name=f"PQ{g}", tag=f"PQ{g}")
            nc.scalar.activation(out=PQ[g], in_=pqp, func=ACT.Copy,
                                 bias=0.0, scale=1.0)

        def qg_make(g):
            ns = group_sizes[g]
            cb = sum(8 * n for n in group_sizes[:g])
            gnbx = CC2[:, cb:cb + 8 * ns].rearrange("p (m s) -> p m s", m=8)
            QG[g] = res.tile([128, 8, ns], F32, name=f"QG{g}", tag=f"QG{g}")
            nc.vector.tensor_tensor(out=QG[g], in0=PQ[g][:, :, 1, :],
                                    in1=gnbx, op=ALU.add)

        VT = [None] * NG

        def apply_affine(g, mos):
            """DVE affine_then_add: V = S3*P + QG + XR per (mo, si)."""
            ns = group_sizes[g]
            if VT[g] is None:
                VT[g] = res.tile([128, 8, ns * PIX], BF16, name=f"VT{g}",
                                 tag=f"VT{g}")
            for mo in mos:
                for si, slot in enumerate(groups[g]):
                    nc.vector.affine_then_add(
                        out=VT[g][:, mo, si * PIX:(si + 1) * PIX],
                        in0=S3[g][:, mo, si * PIX:(si + 1) * PIX],
                        in1=XR[:, mo, slot * PIX:(slot + 1) * PIX],
                        scale=PQ[g][:, mo, 0, si:si + 1],
                        bias=QG[g][:, mo, si:si + 1])

        def apply_relu(g, mos, engine):
            """relu(V) -> OT (one op per mo), DMA out per 4-mo block."""
            ns = group_sizes[g]
            for mo in mos:
                if engine == "dve":
                    nc.vector.tensor_scalar(
                        out=OT[g][:, mo, :], in0=VT[g][:, mo, :],
                        scalar1=0.0, scalar2=None, op0=ALU.max)
                else:
                    nc.scalar.activation(
                        out=OT[g][:, mo, :], in_=VT[g][:, mo, :],
                        func=ACT.Relu, bias=0.0, scale=1.0)
                nc.sync.dma_start(
                    out=out_d.ap()[:, mo,
                                   slot0[g] * PIX:(slot0[g] + ns) * PIX],
                    in_=OT[g][:, mo, :])

        def apply_last(g):
            """Last group (ns==1): DVE affine_then_add from S3 + ACT relu."""
            ns = group_sizes[g]
            slot = groups[g][0]
            for mo in range(8):
                V = rot.tile([128, ns * PIX], BF16, name="V", tag="Vl")
                nc.vector.affine_then_add(
                    out=V,
                    in0=S3[g][:, mo, :],
                    in1=XR[:, mo, slot * PIX:(slot + 1) * PIX],
                    scale=PQ[g][:, mo, 0, 0:1],
                    bias=QG[g][:, mo, 0:1])
                nc.scalar.activation(
                    out=OT[g][:, mo, :], in_=V, func=ACT.Relu,
                    bias=0.0, scale=1.0)
                if mo in (3, 7):
                    nc.sync.dma_start(
                        out=out_d.ap()[:, mo - 3:mo + 1,
                                       slot0[g] * PIX:(slot0[g] + ns) * PIX],
                        in_=OT[g][:, mo - 3:mo + 1, :])

        # ---------------- schedule ----------------
        gl = NG - 1
        if NG == 2:
            conv1(0)
            conv2(0)
            conv3(0)
            Tg0 = stats(0)
            conv1(gl)
            Fv0 = chain(0, Tg0)
            conv2(gl, mos=(0,))
            pq_outer(0, Fv0)
            qg_make(0)
            conv2(gl, mos=(1,))
            apply_affine(0, range(0, 4))
            apply_relu(0, range(0, 4), "act")
            conv3(gl)
            Tg1 = stats(gl)
            Fv1 = chain(gl, Tg1)
            pq_outer(gl, Fv1)
            qg_make(gl)
            apply_last(gl)
            apply_affine(0, range(4, 8))
            apply_relu(0, range(4, 8), "act")
        else:
            # generic fallback (e.g. (2,2) grouping)
            c1_done = set()
            for g in range(NG):
                if g not in c1_done:
                    conv1(g)
                    c1_done.add(g)
                conv2(g)
                conv3(g)
                Tg = stats(g)
                if g == NG - 2:
                    conv1(gl)
                    c1_done.add(gl)
                Fv = chain(g, Tg)
                pq_outer(g, Fv)
                qg_make(g)
                if g == NG - 1 and group_sizes[g] == 1:
                    apply_last(g)
                else:
                    apply_affine(g, range(0, 8))
                    apply_relu(g, range(0, 8), "dve")

    nc.compile()
    return nc


# ----------------------------------------------------------------------------
# Host side
# ----------------------------------------------------------------------------

def _quant_w(w, lv):
    n = max(lv // 2 - 1, 1)
    s = np.float32(np.abs(w).max()) + np.float32(1e-12)
    k = np.round((w.astype(np.float32) / s) * np.float32(n)).astype(np.float32)
    return k, np.float32(s) / np.float32(n)


def _assign_groups(mask):
    mask = np.asarray(mask).astype(np.int64)
    ids = {e: [int(i) for i in np.nonzero(mask == e)[0]] for e in range(3)}
    counts = [len(ids[e]) for e in range(3)]
    if all(c % 2 == 0 for c in counts):
        group_sizes = (2, 2)
        chunks2 = []
        for e in range(3):
            for j in range(0, counts[e], 2):
                chunks2.append((e, ids[e][j:j + 2]))
        assert len(chunks2) == 16
        core_samples = []
        core_experts = []
        for c in range(8):
            (ea, sa), (eb, sb) = chunks2[2 * c], chunks2[2 * c + 1]
            core_samples.append(sa + sb)
            core_experts.append([ea, eb])
        return group_sizes, core_samples, core_experts

    base = [c % 3 for c in counts]
    need = (8 - sum(base)) // 3
    t = [0, 0, 0]
    for e in range(3):
        cap = (counts[e] - base[e]) // 3
        take = min(cap, need)
        t[e] = take
        need -= take
        if need == 0:
            break
    assert need == 0
    b = [base[e] + 3 * t[e] for e in range(3)]
    a = [(counts[e] - b[e]) // 3 for e in range(3)]
    assert sum(a) == 8 and sum(b) == 8
    trip = []
    single = []
    for e in range(3):
        pos = 0
        for _ in range(a[e]):
            trip.append((e, ids[e][pos:pos + 3]))
            pos += 3
        for _ in range(b[e]):
            single.append((e, [ids[e][pos]]))
            pos += 1
        assert pos == counts[e]
    core_samples = []
    core_experts = []
    for c in range(8):
        ea, sa = trip[c]
        eb, sb = single[c]
        core_samples.append(sa + sb)
        core_experts.append([ea, eb])
    return (3, 1), core_samples, core_experts


def kernel(x, mask, w1, w2, w3, bn1_g, bn1_b, bn1_m, bn1_v,
           bn2_g, bn2_b, bn2_m, bn2_v, gn_g, gn_b):
    import ml_dtypes
    from concourse.bass_utils import run_bass_kernel_spmd

    bf16 = ml_dtypes.bfloat16
    f32 = np.float32
    x = np.asarray(x, f32)
    mask = np.asarray(mask)
    w1 = np.asarray(w1, f32)
    w2 = np.asarray(w2, f32)
    w3 = np.asarray(w3, f32)
    bn1 = [np.asarray(v, f32) for v in (bn1_g, bn1_b, bn1_m, bn1_v)]
    bn2 = [np.asarray(v, f32) for v in (bn2_g, bn2_b, bn2_m, bn2_v)]
    gn_g = np.asarray(gn_g, f32)
    gn_b = np.asarray(gn_b, f32)

    group_sizes, core_samples, core_experts = _assign_groups(mask)
    NG = len(group_sizes)

    lv_of = [2 ** b for b in BITS]
    K1, K2, K3 = {}, {}, {}
    CW = {}
    for e in range(3):
        lv = lv_of[e]
        k1, c1 = _quant_w(w1, lv)
        k2, c2 = _quant_w(w2, lv)
        k3, c3 = _quant_w(w3, lv)
        K1[e] = k1.reshape(256, 1024)
        K2[e] = k2.reshape(256, 256, 3, 3)
        K3[e] = k3.reshape(1024, 256)
        CW[e] = (c1, c2, c3)

    inv1 = bn1[0] / np.sqrt(bn1[3] + f32(EPS))
    bb1 = bn1[1] - bn1[2] * inv1
    inv2 = bn2[0] / np.sqrt(bn2[3] + f32(EPS))
    bb2 = bn2[1] - bn2[2] * inv2

    def pack_w(e):
        k1t = K1[e].T.reshape(8, 128, 256).transpose(1, 0, 2)
        k2t = K2[e].transpose(2, 3, 1, 0).reshape(9, 2, 128, 256)
        k2t = k2t.transpose(2, 0, 1, 3)
        k3t = K3[e].T.reshape(2, 128, 1024).transpose(1, 0, 2)
        return (np.ascontiguousarray(k1t).astype(bf16),
                np.ascontiguousarray(k2t).astype(bf16),
                np.ascontiguousarray(k3t).astype(bf16))

    packed = {e: pack_w(e) for e in set(int(v) for v in np.asarray(mask))}

    # host-side input quantization per sample (exact integer grid)
    lv_smp = np.array([lv_of[int(mask[s])] for s in range(B)], f32)
    xq_full = np.clip(np.round(x * (lv_smp - 1)[:, None, None, None]),
                      0.0, (lv_smp - 1)[:, None, None, None]).astype(f32)

    in_maps = []
    for c in range(8):
        sids = core_samples[c]
        experts = core_experts[c]

        # [128, 8, 4*196] channel-tile major
        xqc = xq_full[sids].reshape(4, 8, 128, PIX).transpose(2, 1, 0, 3) \
                           .reshape(128, 8, 4 * PIX)
        xrc = x[sids].reshape(4, 8, 128, PIX).transpose(2, 1, 0, 3) \
                     .reshape(128, 8, 4 * PIX)

        w1c = np.stack([packed[experts[g]][0] for g in range(NG)])
        w2c = np.stack([packed[experts[g]][1] for g in range(NG)])
        w3c = np.stack([packed[experts[g]][2] for g in range(NG)])

        glv = [lv_of[experts[g]] for g in range(NG)]
        NCC = 4 * (2 * NG) + NG + 8 + 8 * NG
        cc = np.zeros((128, NCC), f32)
        a1 = np.zeros((128, 2, NG), f32)
        b1 = np.zeros((128, 2, NG), f32)
        a2 = np.zeros((128, 2, NG), f32)
        b2 = np.zeros((128, 2, NG), f32)
        d3 = np.zeros((128, 8, NG), f32)
        for g in range(NG):
            e = experts[g]
            lv = glv[g]
            c1, c2, c3 = CW[e]
            # offset-128 storage: +128 into the quantizing biases; the
            # 128*rowsum(w) contribution of the offset inputs is removed from
            # the next stage (conv2 bias) or the S3 drain bias (conv3).
            w2sum = K2[e].sum(axis=(1, 2, 3))          # (256,)
            w3sum = K3[e].sum(axis=1)                  # (1024,)
            a1[:, :, g] = (inv1 * c1).reshape(2, 128).T
            b1[:, :, g] = (bb1 * f32(lv - 1)).reshape(2, 128).T + f32(128.0)
            a2[:, :, g] = (inv2 * c2).reshape(2, 128).T
            b2[:, :, g] = (bb2 * f32(lv - 1)
                           - inv2 * c2 * f32(128.0) * w2sum
                           ).reshape(2, 128).T + f32(128.0)
            d3[:, :, g] = (-f32(128.0) * w3sum).reshape(8, 128).T
        o = 0
        cc[:, o:o + 2 * NG] = a1.reshape(128, 2 * NG); o += 2 * NG
        cc[:, o:o + 2 * NG] = b1.reshape(128, 2 * NG); o += 2 * NG
        cc[:, o:o + 2 * NG] = a2.reshape(128, 2 * NG); o += 2 * NG
        cc[:, o:o + 2 * NG] = b2.reshape(128, 2 * NG); o += 2 * NG
        cc[:, o:o + NG] = [128.0 + lv - 1 for lv in glv]; o += NG
        cc[:, o:o + 8] = gn_b.reshape(8, 128).T; o += 8
        cc[:, o:o + 8 * NG] = d3.reshape(128, 8 * NG); o += 8 * NG

        NGR = 1024 + sum(16 * n for n in group_sizes)
        gr = np.zeros((1, NGR), f32)
        gr[0, 0:1024] = gn_g
        off = 1024
        for g in range(NG):
            ns = group_sizes[g]
            e = experts[g]
            lv = glv[g]
            c3e = CW[e][2] / f32(lv - 1)
            gr[0, off:off + 4 * ns] = c3e
            gr[0, off + 4 * ns:off + 8 * ns] = c3e * c3e
            off += 16 * ns

        NC2 = sum(8 * n for n in group_sizes)
        cc2 = np.zeros((128, NC2), f32)
        cb = 0
        gnbp = gn_b.reshape(8, 128).T  # [128, 8]
        for g in range(NG):
            ns = group_sizes[g]
            cc2[:, cb:cb + 8 * ns] = np.repeat(gnbp, ns, axis=1)
            cb += 8 * ns

        in_maps.append({
            "xq": xqc.astype(bf16), "xr": xrc.astype(bf16),
            "w1": w1c, "w2": w2c, "w3": w3c,
            "cc": cc, "gr": gr, "cc2": cc2,
        })

    key = group_sizes
    if key not in _NC_CACHE:
        _NC_CACHE[key] = _build_nc(group_sizes)
    nc = _NC_CACHE[key]

    res = run_bass_kernel_spmd(nc, in_maps, core_ids=list(range(NCORES)))

    out = np.zeros((B, OUTC, H, W), f32)
    for c in range(8):
        oc = np.asarray(res.results[c]["out"], dtype=f32)  # [128, 8, 4*PIX]
        oc = oc.reshape(128, 8, 4, PIX).transpose(2, 1, 0, 3) \
               .reshape(4, OUTC, H, W)
        for t, sid in enumerate(core_samples[c]):
            out[sid] = oc[t]
    return out


# revision 33
# speedup vs baseline: 1.1770x; 1.0155x over previous
"""Trainium2 Bass kernel for quantized-MoE Bottleneck (nn_Bottleneck_37503654429269).

v6 layout:
- Host precomputes quantized activations Xq (bf16 integers) and ships the
  residual x as bf16; no device-side input quantization.
- Offset-128 storage for intermediate quantized activations: the bn affine
  is written by ACT directly as bf16 with +128 folded into the bias, so the
  bf16 output rounding IS the integer rounding; one DVE clamp
  (max 128, min 128+XB) finishes the quantization. Host folds the
  128*rowsum(w) corrections into the next stage's bias (conv2) or the S3
  drain bias (conv3).
- Group-major schedule: g0 conv1->2->3 completes early; its GN apply
  overlaps g1's convs. All conv3 outputs drain to S3 sbuf (bf16) with the
  offset correction applied; bn_stats reads S3 so PSUM recycles fast.
- DMA: priority-ordered on the SP queue (W1/XQ of g0 first); late tensors
  (XR, g1 conv2/conv3 weights) issue from the idle GpSimd queue.
"""

import numpy as np

BITS = (2, 4, 8)
EPS = 1e-5
B, C_IN, H, W = 32, 1024, 14, 14
WIDTH, OUTC = 256, 1024
PIX = H * W  # 196
NCORES = 8
RB = float(2.0 ** 23)

_NC_CACHE = {}


# ----------------------------------------------------------------------------
# Device program
# ----------------------------------------------------------------------------

def _build_nc(group_sizes):
    from contextlib import ExitStack
    import concourse.bacc as bacc
    import concourse.mybir as mybir
    import concourse.tile as tile

    F32 = mybir.dt.float32
    BF16 = mybir.dt.bfloat16
    ALU = mybir.AluOpType
    ACT = mybir.ActivationFunctionType

    NG = len(group_sizes)
    NS = sum(group_sizes)
    assert NS == 4
    slot0 = [sum(group_sizes[:g]) for g in range(NG)]
    groups = [list(range(slot0[g], slot0[g] + group_sizes[g])) for g in range(NG)]
    chunks = {g: [groups[g][i:i + 2] for i in range(0, len(groups[g]), 2)]
              for g in range(NG)}
    # last group should be the smallest (shortest tail)
    NSL = group_sizes[-1]

    nc = bacc.Bacc("TRN2", target_bir_lowering=False, debug=False,
                   num_devices=NCORES)

    # ---- dram tensors
    # xq: quantized activations [128, kt(8), 4*196] bf16 (integers)
    xq_d = nc.dram_tensor("xq", [128, 8, 4 * PIX], BF16, kind="ExternalInput")
    # xr: residual x [128, mo(8), 4*196] bf16
    xr_d = nc.dram_tensor("xr", [128, 8, 4 * PIX], BF16, kind="ExternalInput")
    w1_d = nc.dram_tensor("w1", [NG, 128, 8, 256], BF16, kind="ExternalInput")
    w2_d = nc.dram_tensor("w2", [NG, 128, 9, 2, 256], BF16, kind="ExternalInput")
    w3_d = nc.dram_tensor("w3", [NG, 128, 2, 1024], BF16, kind="ExternalInput")
    # packed per-partition consts:
    # a1[2,NG] b1r[2,NG] a2[2,NG] b2r[2,NG] xb[NG] gnb[8] d3[8,NG]
    NCC = 4 * (2 * NG) + NG + 8 + 8 * NG
    cc_d = nc.dram_tensor("cc", [128, NCC], F32, kind="ExternalInput")
    # row consts: gng[1024] + per-g (c3e[4*ns], c3e2[4*ns]) + gnbx[8*ns per g]
    NGR = 1024 + sum(16 * n for n in group_sizes)
    gr_d = nc.dram_tensor("gr", [1, NGR], F32, kind="ExternalInput")
    out_d = nc.dram_tensor("out", [128, 8, 4 * PIX], BF16, kind="ExternalOutput")

    with tile.TileContext(nc) as tc, ExitStack() as ctx:
        res = ctx.enter_context(tc.tile_pool(name="res", bufs=1))
        rot = ctx.enter_context(tc.tile_pool(name="rot", bufs=4))
        mmp = ctx.enter_context(tc.tile_pool(name="mmp", bufs=5, space="PSUM"))
        smp = ctx.enter_context(tc.tile_pool(name="smp", bufs=1, space="PSUM"))
        pqq = ctx.enter_context(tc.tile_pool(name="pqq", bufs=2, space="PSUM"))

        # ---- persistent tiles
        CC = res.tile([128, NCC], F32, name="CC", tag="CC")
        o = 0
        A1 = CC[:, o:o + 2 * NG].rearrange("p (m g) -> p m g", m=2); o += 2 * NG
        B1R = CC[:, o:o + 2 * NG].rearrange("p (m g) -> p m g", m=2); o += 2 * NG
        A2 = CC[:, o:o + 2 * NG].rearrange("p (m g) -> p m g", m=2); o += 2 * NG
        B2R = CC[:, o:o + 2 * NG].rearrange("p (m g) -> p m g", m=2); o += 2 * NG
        XB = CC[:, o:o + NG]; o += NG
        GNB = CC[:, o:o + 8]; o += 8
        D3 = CC[:, o:o + 8 * NG].rearrange("p (m g) -> p m g", m=8); o += 8 * NG

        GR = res.tile([1, NGR], F32, name="GR", tag="GR")
        GNG = GR[:, 0:1024]

        # gnbx: per-partition gn_b replicated per sample, [128, 8*ns] per group
        NC2 = sum(8 * n for n in group_sizes)
        cc2_d = nc.dram_tensor("cc2", [128, NC2], F32, kind="ExternalInput")
        CC2 = res.tile([128, NC2], F32, name="CC2", tag="CC2")

        XQ = res.tile([128, 8, 4 * PIX], BF16, name="XQ", tag="XQ")
        XR = res.tile([128, 8, 4 * PIX], BF16, name="XR", tag="XR")
        W1 = [res.tile([128, 8, 256], BF16, name=f"W1_{g}", tag=f"W1_{g}")
              for g in range(NG)]
        W2 = [res.tile([128, 9, 2, 256], BF16, name=f"W2_{g}", tag=f"W2_{g}")
              for g in range(NG)]
        W3 = [res.tile([128, 2, 1024], BF16, name=f"W3_{g}", tag=f"W3_{g}")
              for g in range(NG)]

        # ---- DMA: single SP queue in strict need-order; XQ-g0 split per
        # kt-pair so conv1's psum accumulation can start on the first pair.
        n0 = group_sizes[0] * PIX
        # first-needed tensors issue from the scalar/vector queues, which are
        # idle at kernel start: their transfers begin ~2us before SP's.
        nc.scalar.dma_start(out=W1[0], in_=w1_d.ap()[0])
        nc.scalar.dma_start(out=XQ[:, 0:2, 0:n0],
                            in_=xq_d.ap()[:, 0:2, 0:n0])
        nc.gpsimd.dma_start(out=XQ[:, 2:4, 0:n0],
                            in_=xq_d.ap()[:, 2:4, 0:n0])
        nc.sync.dma_start(out=CC, in_=cc_d.ap())
        nc.sync.dma_start(out=CC2, in_=cc2_d.ap())
        nc.sync.dma_start(out=GR, in_=gr_d.ap())
        for kp in range(2, 4):
            nc.sync.dma_start(out=XQ[:, 2 * kp:2 * kp + 2, 0:n0],
                              in_=xq_d.ap()[:, 2 * kp:2 * kp + 2, 0:n0])
        nc.sync.dma_start(out=W2[0], in_=w2_d.ap()[0])
        nc.sync.dma_start(out=W3[0], in_=w3_d.ap()[0])
        for g in range(1, NG):
            nc.sync.dma_start(
                out=XQ[:, :, slot0[g] * PIX:(slot0[g] + group_sizes[g]) * PIX],
                in_=xq_d.ap()[:, :, slot0[g] * PIX:(slot0[g] + group_sizes[g]) * PIX])
            nc.sync.dma_start(out=W1[g], in_=w1_d.ap()[g])
        nc.sync.dma_start(out=XR, in_=xr_d.ap())
        for g in range(1, NG):
            nc.sync.dma_start(out=W2[g], in_=w2_d.ap()[g])
            nc.sync.dma_start(out=W3[g], in_=w3_d.ap()[g])

        ONES = res.tile([128, 1], F32, name="ONES", tag="ONES")
        nc.vector.memset(ONES, 1.0)

        # HP padded conv2 inputs, zero ring (Pool memsets, early)
        HP = [[res.tile([128, group_sizes[g], 16, 18], BF16,
                        name=f"HP{kt}_{g}", tag=f"HP{kt}_{g}")
               for g in range(NG)] for kt in range(2)]
        for kt in range(2):
            for g in range(NG):
                nc.gpsimd.memset(HP[kt][g], 128.0)

        Q2 = [[res.tile([128, group_sizes[g] * PIX], BF16,
                        name=f"Q2{kt}_{g}", tag=f"Q2{kt}_{g}")
               for g in range(NG)] for kt in range(2)]
        # S3 sbuf (bf16, offset-corrected conv3 output) for all groups
        S3 = [res.tile([128, 8, group_sizes[g] * PIX], BF16,
                       name=f"S3_{g}", tag=f"S3_{g}")
              for g in range(NG)]

        BST = [res.tile([128, 8 * group_sizes[g] * 8], F32, name=f"BST{g}",
                        tag=f"BST{g}") for g in range(NG)]
        PQ = [None] * NG
        QG = [None] * NG
        OT = [res.tile([128, 8, group_sizes[g] * PIX], BF16,
                       name=f"OT{g}", tag=f"OT{g}") for g in range(NG)]

        def c1_post(g, mo, ch, ps, pool_rr=None):
            nch = len(ch)
            c0 = ch[0] - slot0[g]
            # bf16 store of a*ps + b + 128 rounds to the integer grid in
            # [128, 256); one clamp finishes quantization (offset-128 kept).
            tpr = rot.tile([128, nch * PIX], BF16, name="tpr", tag="tpr")
            nc.scalar.activation(out=tpr, in_=ps, func=ACT.Identity,
                                 bias=B1R[:, mo, g:g + 1],
                                 scale=A1[:, mo, g:g + 1])
            nc.vector.tensor_scalar(
                out=HP[mo][g][:, c0:c0 + nch, 1:15, 2:16],
                in0=tpr.rearrange("p (s y x) -> p s y x", s=nch, y=14),
                scalar1=128.0, scalar2=XB[:, g:g + 1],
                op0=ALU.max, op1=ALU.min)

        def conv1(g, pool_rr=True):
            for mo in range(2):
                for ch in chunks[g]:
                    nch = len(ch)
                    ps = mmp.tile([128, nch * PIX], F32, name="c1ps", tag="mm")
                    for kt in range(8):
                        nc.tensor.matmul(
                            ps,
                            W1[g][:, kt, mo * 128:(mo + 1) * 128],
                            XQ[:, kt, ch[0] * PIX:(ch[0] + nch) * PIX],
                            start=(kt == 0), stop=(kt == 7))
                    c1_post(g, mo, ch, ps, pool_rr)

        def c2_post(g, mo, ch, ps, pool_rr=None):
            nch = len(ch)
            c0 = ch[0] - slot0[g]
            tpr = rot.tile([128, nch * PIX], BF16, name="tpr", tag="tpr")
            nc.scalar.activation(
                out=tpr, in_=ps.rearrange("p s y x -> p (s y x)"),
                func=ACT.Identity,
                bias=B2R[:, mo, g:g + 1], scale=A2[:, mo, g:g + 1])
            nc.vector.tensor_scalar(
                out=Q2[mo][g][:, c0 * PIX:(c0 + nch) * PIX],
                in0=tpr, scalar1=128.0, scalar2=XB[:, g:g + 1],
                op0=ALU.max, op1=ALU.min)

        def conv2(g, pool_rr=True, defer_posts=False, mos=(0, 1)):
            posts = []
            for mo in mos:
                for ch in chunks[g]:
                    nch = len(ch)
                    c0 = ch[0] - slot0[g]
                    ps = mmp.tile([128, nch, 14, 14], F32, name="c2ps",
                                  tag="mm")
                    first = True
                    for ti, (dy, dx) in enumerate(
                            (dy, dx) for dy in range(3) for dx in range(3)):
                        for kt in range(2):
                            nc.tensor.matmul(
                                ps,
                                W2[g][:, ti, kt, mo * 128:(mo + 1) * 128],
                                HP[kt][g][:, c0:c0 + nch,
                                          dy:dy + 14, dx + 1:dx + 15],
                                start=first, stop=(ti == 8 and kt == 1))
                            first = False
                    if defer_posts:
                        posts.append((mo, ch, ps))
                    else:
                        c2_post(g, mo, ch, ps, pool_rr)
            return posts

        def conv3(g):
            """psum -> ACT drain to S3 sbuf bf16 (removing the 128-offset
            contribution via the -D3 bias); bn_stats reads S3."""
            ns = group_sizes[g]
            bstv = BST[g][:, 0:8 * ns * 6].rearrange("p (t c) -> p t c", c=6)
            for mo in range(8):
                for ch in chunks[g]:
                    nch = len(ch)
                    c0 = ch[0] - slot0[g]
                    ps = mmp.tile([128, nch * PIX], F32, name="c3ps", tag="mm")
                    for kt in range(2):
                        nc.tensor.matmul(
                            ps,
                            W3[g][:, kt, mo * 128:(mo + 1) * 128],
                            Q2[kt][g][:, c0 * PIX:(c0 + nch) * PIX],
                            start=(kt == 0), stop=(kt == 1))
                    nc.scalar.activation(
                        out=S3[g][:, mo, c0 * PIX:(c0 + nch) * PIX],
                        in_=ps, func=ACT.Identity,
                        bias=D3[:, mo, g:g + 1], scale=1.0)
                for si in range(ns):
                    nc.vector.bn_stats(
                        out=bstv[:, mo * ns + si:mo * ns + si + 1, :],
                        in_=S3[g][:, mo, si * PIX:(si + 1) * PIX])

        def stats(g):
            ns = group_sizes[g]
            nst = 8 * ns
            # mean^2 columns (cols 1 and 4 of each 6-tuple)
            mvi = BST[g][:, 0:nst * 6].rearrange(
                "p (t h c) -> p t h c", h=2, c=3)[:, :, :, 1]
            msq = BST[g][:, nst * 6:nst * 8].rearrange("p (t h) -> p t h", h=2)
            nc.vector.tensor_tensor(out=msq, in0=mvi, in1=mvi, op=ALU.mult)
            # partition reduce
            red = smp.tile([1, nst * 8], F32, name="red", tag="red")
            nc.tensor.matmul(red, ONES, BST[g], start=True, stop=True)
            Tg = res.tile([1, nst * 8], F32, name=f"T{g}", tag=f"T{g}")
            nc.scalar.activation(out=Tg, in_=red, func=ACT.Copy,
                                 bias=0.0, scale=1.0)
            return Tg

        def chain(g, Tg):
            """mo-parity pair-add + scalar math -> Fv [1, 8*ns]."""
            ns = group_sizes[g]
            nst = 8 * ns
            nsc = 4 * ns
            TB = res.tile([1, 4 * ns * 8], F32, name=f"TB{g}", tag=f"TB{g}")
            tv = Tg[:, 0:nst * 6].rearrange("p (m o s c) -> p m o s c",
                                            m=4, o=2, c=6)
            nc.vector.tensor_tensor(
                out=TB[:, 0:4 * ns * 6].rearrange("p (m s c) -> p m s c",
                                                  m=4, c=6),
                in0=tv[:, :, 0, :, :], in1=tv[:, :, 1, :, :], op=ALU.add)
            mv = Tg[:, nst * 6:nst * 8].rearrange("p (m o s c) -> p m o s c",
                                                  m=4, o=2, c=2)
            nc.vector.tensor_tensor(
                out=TB[:, 4 * ns * 6:4 * ns * 8].rearrange(
                    "p (m s c) -> p m s c", m=4, c=2),
                in0=mv[:, :, 0, :, :], in1=mv[:, :, 1, :, :], op=ALU.add)
            tb6 = TB[:, 0:4 * ns * 6].rearrange("p (t c) -> p t c", c=6)
            tb2 = TB[:, 4 * ns * 6:4 * ns * 8].rearrange("p (t c) -> p t c",
                                                         c=2)
            SC = res.tile([1, nsc * 4], F32, name=f"SC{g}", tag=f"SC{g}")
            scv = SC.rearrange("p (c t) -> p c t", c=4)
            nc.vector.tensor_tensor(out=scv[:, 0, :], in0=tb6[:, :, 1],
                                    in1=tb6[:, :, 4], op=ALU.add)
            nc.vector.tensor_tensor(out=scv[:, 1, :], in0=tb6[:, :, 2],
                                    in1=tb6[:, :, 5], op=ALU.add)
            nc.vector.tensor_tensor(out=scv[:, 2, :], in0=tb2[:, :, 0],
                                    in1=tb2[:, :, 1], op=ALU.add)
            MEAN = rot.tile([1, nsc], F32, name="MEAN", tag=f"MEAN{g}")
            nc.vector.tensor_scalar(out=MEAN, in0=scv[:, 0, :],
                                    scalar1=1.0 / 512, scalar2=None,
                                    op0=ALU.mult)
            E2 = rot.tile([1, nsc], F32, name="E2", tag=f"E2{g}")
            nc.vector.scalar_tensor_tensor(out=E2, in0=scv[:, 2, :],
                                           scalar=98.0, in1=scv[:, 1, :],
                                           op0=ALU.mult, op1=ALU.add)
            nc.vector.tensor_scalar(out=E2, in0=E2,
                                    scalar1=1.0 / (2 * 128 * PIX),
                                    scalar2=None, op0=ALU.mult)
            VAR = rot.tile([1, nsc], F32, name="VAR", tag=f"VAR{g}")
            nc.vector.tensor_tensor(out=VAR, in0=MEAN, in1=MEAN, op=ALU.mult)
            nc.vector.tensor_tensor(out=VAR, in0=E2, in1=VAR, op=ALU.subtract)
            cbase = 1024 + sum(16 * n for n in group_sizes[:g])
            nc.vector.tensor_tensor(out=VAR, in0=VAR,
                                    in1=GR[:, cbase + nsc:cbase + 2 * nsc],
                                    op=ALU.mult)
            nc.vector.tensor_scalar(out=VAR, in0=VAR, scalar1=EPS,
                                    scalar2=None, op0=ALU.add)
            SD = rot.tile([1, nsc], F32, name="SD", tag=f"SD{g}")
            nc.scalar.activation(out=SD, in_=VAR, func=ACT.Sqrt,
                                 bias=0.0, scale=1.0)
            RC = rot.tile([1, nsc], F32, name="RC", tag=f"RC{g}")
            nc.vector.reciprocal(out=RC, in_=SD)
            Fv = res.tile([1, 8 * ns], F32, name=f"F_{g}", tag=f"F_{g}")
            nc.vector.tensor_tensor(out=Fv[:, 0:nsc], in0=RC,
                                    in1=GR[:, cbase:cbase + nsc], op=ALU.mult)
            nc.vector.scalar_tensor_tensor(
                out=Fv[:, nsc:2 * nsc], in0=MEAN, scalar=-1.0,
                in1=Fv[:, 0:nsc], op0=ALU.mult, op1=ALU.mult)
            return Fv

        def pq_outer(g, Fv):
            """P,Q outer products on PE; ACT drain."""
            ns = group_sizes[g]
            pqp = pqq.tile([128, 8, 2, ns], F32, name="pqp", tag="pqp")
            fvv = Fv.rearrange("p (k m s) -> p k m s", k=2, m=4)
            for mo in range(8):
                nc.tensor.matmul(
                    pqp[:, mo, :, :],
                    GNG[:, mo * 128:(mo + 1) * 128],
                    fvv[:, :, mo // 2, :],
                    start=(mo == 0), stop=(mo == 7), skip_group_check=True)
            PQ[g] = pqp

        def qg_make(g):
            ns = group_sizes[g]
            cb = sum(8 * n for n in group_sizes[:g])
            gnbx = CC2[:, cb:cb + 8 * ns].rearrange("p (m s) -> p m s", m=8)
            QG[g] = res.tile([128, 8, ns], F32, name=f"QG{g}", tag=f"QG{g}")
            nc.vector.tensor_tensor(out=QG[g], in0=PQ[g][:, :, 1, :],
                                    in1=gnbx, op=ALU.add)

        VT = [None] * NG

        def apply_affine(g, mos):
            """DVE affine_then_add: V = S3*P + QG + XR per (mo, si)."""
            ns = group_sizes[g]
            if VT[g] is None:
                VT[g] = res.tile([128, 8, ns * PIX], BF16, name=f"VT{g}",
                                 tag=f"VT{g}")
            for mo in mos:
                for si, slot in enumerate(groups[g]):
                    nc.vector.affine_then_add(
                        out=VT[g][:, mo, si * PIX:(si + 1) * PIX],
                        in0=S3[g][:, mo, si * PIX:(si + 1) * PIX],
                        in1=XR[:, mo, slot * PIX:(slot + 1) * PIX],
                        scale=PQ[g][:, mo, 0, si:si + 1],
                        bias=QG[g][:, mo, si:si + 1])

        def apply_relu(g, mos, engine):
            """relu(V) -> OT (one op per mo), DMA out per 4-mo block."""
            ns = group_sizes[g]
            for mo in mos:
                if engine == "dve":
                    nc.vector.tensor_scalar(
                        out=OT[g][:, mo, :], in0=VT[g][:, mo, :],
                        scalar1=0.0, scalar2=None, op0=ALU.max)
                else:
                    nc.scalar.activation(
                        out=OT[g][:, mo, :], in_=VT[g][:, mo, :],
                        func=ACT.Relu, bias=0.0, scale=1.0)
                nc.sync.dma_start(
                    out=out_d.ap()[:, mo,
                                   slot0[g] * PIX:(slot0[g] + ns) * PIX],
                    in_=OT[g][:, mo, :])

        def apply_last(g):
            """Last group (ns==1): DVE affine_then_add from S3 + ACT relu."""
            ns = group_sizes[g]
            slot = groups[g][0]
            for mo in range(8):
                V = rot.tile([128, ns * PIX], BF16, name="V", tag="Vl")
                nc.vector.affine_then_add(
                    out=V,
                    in0=S3[g][:, mo, :],
                    in1=XR[:, mo, slot * PIX:(slot + 1) * PIX],
                    scale=PQ[g][:, mo, 0, 0:1],
                    bias=QG[g][:, mo, 0:1])
                nc.scalar.activation(
                    out=OT[g][:, mo, :], in_=V, func=ACT.Relu,
                    bias=0.0, scale=1.0)
                if mo in (3, 7):
                    nc.sync.dma_start(
                        out=out_d.ap()[:, mo - 3:mo + 1,
                                       slot0[g] * PIX:(slot0[g] + ns) * PIX],
                        in_=OT[g][:, mo - 3:mo + 1, :])

        # ---------------- schedule ----------------
        gl = NG - 1
        if NG == 2:
            conv1(0)
            conv2(0)
            conv3(0)
            Tg0 = stats(0)
            conv1(gl)
            Fv0 = chain(0, Tg0)
            conv2(gl, mos=(0,))
            pq_outer(0, Fv0)
            qg_make(0)
            conv2(gl, mos=(1,))
            apply_affine(0, range(0, 4))
            conv3(gl)
            apply_relu(0, range(0, 4), "act")
            Tg1 = stats(gl)
            Fv1 = chain(gl, Tg1)
            pq_outer(gl, Fv1)
            qg_make(gl)
            apply_last(gl)
            apply_affine(0, range(4, 8))
            apply_relu(0, range(4, 8), "act")
        else:
            # generic fallback (e.g. (2,2) grouping)
            c1_done = set()
            for g in range(NG):
                if g not in c1_done:
                    conv1(g)
                    c1_done.add(g)
                conv2(g)
                conv3(g)
                Tg = stats(g)
                if g == NG - 2:
                    conv1(gl)
                    c1_done.add(gl)
                Fv = chain(g, Tg)
                pq_outer(g, Fv)
                qg_make(g)
                if g == NG - 1 and group_sizes[g] == 1:
                    apply_last(g)
                else:
                    apply_affine(g, range(0, 8))
                    apply_relu(g, range(0, 8), "dve")

    nc.compile()
    return nc


# ----------------------------------------------------------------------------
# Host side
# ----------------------------------------------------------------------------

def _quant_w(w, lv):
    n = max(lv // 2 - 1, 1)
    s = np.float32(np.abs(w).max()) + np.float32(1e-12)
    k = np.round((w.astype(np.float32) / s) * np.float32(n)).astype(np.float32)
    return k, np.float32(s) / np.float32(n)


def _assign_groups(mask):
    mask = np.asarray(mask).astype(np.int64)
    ids = {e: [int(i) for i in np.nonzero(mask == e)[0]] for e in range(3)}
    counts = [len(ids[e]) for e in range(3)]
    if all(c % 2 == 0 for c in counts):
        group_sizes = (2, 2)
        chunks2 = []
        for e in range(3):
            for j in range(0, counts[e], 2):
                chunks2.append((e, ids[e][j:j + 2]))
        assert len(chunks2) == 16
        core_samples = []
        core_experts = []
        for c in range(8):
            (ea, sa), (eb, sb) = chunks2[2 * c], chunks2[2 * c + 1]
            core_samples.append(sa + sb)
            core_experts.append([ea, eb])
        return group_sizes, core_samples, core_experts

    base = [c % 3 for c in counts]
    need = (8 - sum(base)) // 3
    t = [0, 0, 0]
    for e in range(3):
        cap = (counts[e] - base[e]) // 3
        take = min(cap, need)
        t[e] = take
        need -= take
        if need == 0:
            break
    assert need == 0
    b = [base[e] + 3 * t[e] for e in range(3)]
    a = [(counts[e] - b[e]) // 3 for e in range(3)]
    assert sum(a) == 8 and sum(b) == 8
    trip = []
    single = []
    for e in range(3):
        pos = 0
        for _ in range(a[e]):
            trip.append((e, ids[e][pos:pos + 3]))
            pos += 3
        for _ in range(b[e]):
            single.append((e, [ids[e][pos]]))
            pos += 1
        assert pos == counts[e]
    core_samples = []
    core_experts = []
    for c in range(8):
        ea, sa = trip[c]
        eb, sb = single[c]
        core_samples.append(sa + sb)
        core_experts.append([ea, eb])
    return (3, 1), core_samples, core_experts


def kernel(x, mask, w1, w2, w3, bn1_g, bn1_b, bn1_m, bn1_v,
           bn2_g, bn2_b, bn2_m, bn2_v, gn_g, gn_b):
    import ml_dtypes
    from concourse.bass_utils import run_bass_kernel_spmd

    bf16 = ml_dtypes.bfloat16
    f32 = np.float32
    x = np.asarray(x, f32)
    mask = np.asarray(mask)
    w1 = np.asarray(w1, f32)
    w2 = np.asarray(w2, f32)
    w3 = np.asarray(w3, f32)
    bn1 = [np.asarray(v, f32) for v in (bn1_g, bn1_b, bn1_m, bn1_v)]
    bn2 = [np.asarray(v, f32) for v in (bn2_g, bn2_b, bn2_m, bn2_v)]
    gn_g = np.asarray(gn_g, f32)
    gn_b = np.asarray(gn_b, f32)

    group_sizes, core_samples, core_experts = _assign_groups(mask)
    NG = len(group_sizes)

    lv_of = [2 ** b for b in BITS]
    K1, K2, K3 = {}, {}, {}
    CW = {}
    for e in range(3):
        lv = lv_of[e]
        k1, c1 = _quant_w(w1, lv)
        k2, c2 = _quant_w(w2, lv)
        k3, c3 = _quant_w(w3, lv)
        K1[e] = k1.reshape(256, 1024)
        K2[e] = k2.reshape(256, 256, 3, 3)
        K3[e] = k3.reshape(1024, 256)
        CW[e] = (c1, c2, c3)

    inv1 = bn1[0] / np.sqrt(bn1[3] + f32(EPS))
    bb1 = bn1[1] - bn1[2] * inv1
    inv2 = bn2[0] / np.sqrt(bn2[3] + f32(EPS))
    bb2 = bn2[1] - bn2[2] * inv2

    def pack_w(e):
        k1t = K1[e].T.reshape(8, 128, 256).transpose(1, 0, 2)
        k2t = K2[e].transpose(2, 3, 1, 0).reshape(9, 2, 128, 256)
        k2t = k2t.transpose(2, 0, 1, 3)
        k3t = K3[e].T.reshape(2, 128, 1024).transpose(1, 0, 2)
        return (np.ascontiguousarray(k1t).astype(bf16),
                np.ascontiguousarray(k2t).astype(bf16),
                np.ascontiguousarray(k3t).astype(bf16))

    packed = {e: pack_w(e) for e in set(int(v) for v in np.asarray(mask))}

    # host-side input quantization per sample (exact integer grid)
    lv_smp = np.array([lv_of[int(mask[s])] for s in range(B)], f32)
    xq_full = np.clip(np.round(x * (lv_smp - 1)[:, None, None, None]),
                      0.0, (lv_smp - 1)[:, None, None, None]).astype(f32)

    in_maps = []
    for c in range(8):
        sids = core_samples[c]
        experts = core_experts[c]

        # [128, 8, 4*196] channel-tile major
        xqc = xq_full[sids].reshape(4, 8, 128, PIX).transpose(2, 1, 0, 3) \
                           .reshape(128, 8, 4 * PIX)
        xrc = x[sids].reshape(4, 8, 128, PIX).transpose(2, 1, 0, 3) \
                     .reshape(128, 8, 4 * PIX)

        w1c = np.stack([packed[experts[g]][0] for g in range(NG)])
        w2c = np.stack([packed[experts[g]][1] for g in range(NG)])
        w3c = np.stack([packed[experts[g]][2] for g in range(NG)])

        glv = [lv_of[experts[g]] for g in range(NG)]
        NCC = 4 * (2 * NG) + NG + 8 + 8 * NG
        cc = np.zeros((128, NCC), f32)
        a1 = np.zeros((128, 2, NG), f32)
        b1 = np.zeros((128, 2, NG), f32)
        a2 = np.zeros((128, 2, NG), f32)
        b2 = np.zeros((128, 2, NG), f32)
        d3 = np.zeros((128, 8, NG), f32)
        for g in range(NG):
            e = experts[g]
            lv = glv[g]
            c1, c2, c3 = CW[e]
            # offset-128 storage: +128 into the quantizing biases; the
            # 128*rowsum(w) contribution of the offset inputs is removed from
            # the next stage (conv2 bias) or the S3 drain bias (conv3).
            w2sum = K2[e].sum(axis=(1, 2, 3))          # (256,)
            w3sum = K3[e].sum(axis=1)                  # (1024,)
            a1[:, :, g] = (inv1 * c1).reshape(2, 128).T
            b1[:, :, g] = (bb1 * f32(lv - 1)).reshape(2, 128).T + f32(128.0)
            a2[:, :, g] = (inv2 * c2).reshape(2, 128).T
            b2[:, :, g] = (bb2 * f32(lv - 1)
                           - inv2 * c2 * f32(128.0) * w2sum
                           ).reshape(2, 128).T + f32(128.0)
            d3[:, :, g] = (-f32(128.0) * w3sum).reshape(8, 128).T
        o = 0
        cc[:, o:o + 2 * NG] = a1.reshape(128, 2 * NG); o += 2 * NG
        cc[:, o:o + 2 * NG] = b1.reshape(128, 2 * NG); o += 2 * NG
        cc[:, o:o + 2 * NG] = a2.reshape(128, 2 * NG); o += 2 * NG
        cc[:, o:o + 2 * NG] = b2.reshape(128, 2 * NG); o += 2 * NG
        cc[:, o:o + NG] = [128.0 + lv - 1 for lv in glv]; o += NG
        cc[:, o:o + 8] = gn_b.reshape(8, 128).T; o += 8
        cc[:, o:o + 8 * NG] = d3.reshape(128, 8 * NG); o += 8 * NG

        NGR = 1024 + sum(16 * n for n in group_sizes)
        gr = np.zeros((1, NGR), f32)
        gr[0, 0:1024] = gn_g
        off = 1024
        for g in range(NG):
            ns = group_sizes[g]
            e = experts[g]
            lv = glv[g]
            c3e = CW[e][2] / f32(lv - 1)
            gr[0, off:off + 4 * ns] = c3e
            gr[0, off + 4 * ns:off + 8 * ns] = c3e * c3e
            off += 16 * ns

        NC2 = sum(8 * n for n in group_sizes)
        cc2 = np.zeros((128, NC2), f32)
        cb = 0
        gnbp = gn_b.reshape(8, 128).T  # [128, 8]
        for g in range(NG):
            ns = group_sizes[g]
            cc2[:, cb:cb + 8 * ns] = np.repeat(gnbp, ns, axis=1)
            cb += 8 * ns

        in_maps.append({
            "xq": xqc.astype(bf16), "xr": xrc.astype(bf16),
            "w1": w1c, "w2": w2c, "w3": w3c,
            "cc": cc, "gr": gr, "cc2": cc2,
        })

    key = group_sizes
    if key not in _NC_CACHE:
        _NC_CACHE[key] = _build_nc(group_sizes)
    nc = _NC_CACHE[key]

    res = run_bass_kernel_spmd(nc, in_maps, core_ids=list(range(NCORES)))

    out = np.zeros((B, OUTC, H, W), f32)
    for c in range(8):
        oc = np.asarray(res.results[c]["out"], dtype=f32)  # [128, 8, 4*PIX]
        oc = oc.reshape(128, 8, 4, PIX).transpose(2, 1, 0, 3) \
               .reshape(4, OUTC, H, W)
        for t, sid in enumerate(core_samples[c]):
            out[sid] = oc[t]
    return out
